# revision 1
# baseline (speedup 1.0000x reference)
# Trainium2 Bass SPMD kernel for nn_MultiHeadAttn_16492674416882.
#
# kernel(**inputs) takes the FULL fp32 inputs and returns the FULL
# (B, D, S) output, running a fused per-core program on 8 NeuronCores.
#
# Sharding: core i handles batch b=i//2 and query-half h=i%2 (1024 of the
# 2048 positions). K/V projections for a batch are computed by both cores of
# the pair (cheap duplication), which removes every large collective; the
# only cross-core communication is an 8KB AllReduce of BatchNorm statistics.
#
# Per-core device dataflow (all matmuls bf16 with fp32 PSUM accumulation):
#   qT[k,s] = wq.T @ Q^T    kT[k,t] = wk.T @ K^T    v[t,c] = V^T.T @ wv
#   v_aug per head pair: [V_A(64) | ones(1) | gap(63) | V_B(64)]
#   scores^T[t,s] = kT_h.T @ qT_h   (two heads row-tiled on the PE, K=64)
#   p = exp(scores * SCALE)         (ACT with fused scale, bf16 out)
#   oA = vaugA.T @ pA  -> rows 0:64 attn sums, row 64 = softmax denominator
#   oB = vaugB.T @ pB  -> row 0 = denominator, rows 64:128 attn sums
#   concat = o * bcast(1/den)       (reciprocal_approx_fast + partition_broadcast)
#   outT[d,s] = wo.T @ concat + bo + Q^T (residual, exact via bf16 hi+lo)
#   BN: local sum/sumsq -> AllReduce over 8 cores -> normalize -> out [D, 1024]
#
# The residual is kept fp32-exact by shipping Q^T as two bf16 tensors
# (hi = bf16(q), lo = bf16(q - hi)); all attention-path bf16 rounding is
# damped ~50x in the final output because the residual dominates it.

import math
import os
import sys
from contextlib import ExitStack
from dataclasses import dataclass

import numpy as np
import ml_dtypes

for _p in ("/root/.axon_site/_ro/trn_rl_repo", "/opt/trn_rl_repo"):
    if _p not in sys.path and os.path.isdir(_p):
        sys.path.append(_p)

import concourse.bass as bass
import concourse.tile as tile
from concourse import bacc, mybir, library_config
from concourse.bass import ds, ts
from concourse.bass_utils import run_bass_kernel_spmd

F32 = mybir.dt.float32
BF16 = mybir.dt.bfloat16
AF = mybir.ActivationFunctionType
ALU = mybir.AluOpType
BF = ml_dtypes.bfloat16


@dataclass
class Cfg:
    D: int = 1024          # model dim (== H*64)
    H: int = 16            # heads
    S_HALF: int = 1024     # queries per core
    T: int = 2048          # kv length
    n_cores: int = 8
    n_total: int = 8192    # BN normalization count (B*S)
    use_collective: bool = True
    phase_limit: int = 3   # debug: 1=projections, 2=+attention, 3=full
    eps: float = 1e-5
    scale: float = math.sqrt(1.0 / 1024.0)

    @property
    def ND(self): return self.D // 128          # d chunks
    @property
    def NPAIR(self): return self.H // 2         # head pairs
    @property
    def TCK(self): return self.T // 128         # t chunks for attention
    @property
    def SC(self): return min(512, self.S_HALF)  # s chunk
    @property
    def NSC(self): return self.S_HALF // self.SC
    @property
    def KC(self): return min(512, self.T)       # k-proj t chunk
    @property
    def NKC(self): return self.T // self.KC
    @property
    def HV(self): return self.H * 64            # total v columns
    @property
    def VC(self): return min(512, self.HV)      # v-proj col chunk
    @property
    def NVC(self): return self.HV // self.VC
    @property
    def PAIRW(self): return 192                 # v_aug per-pair width


def build_program(cfg: Cfg) -> bass.Bass:
    nc = bacc.Bacc("TRN2", target_bir_lowering=False, debug=False,
                   num_devices=cfg.n_cores)
    D, H, SH, T = cfg.D, cfg.H, cfg.S_HALF, cfg.T
    ND, NPAIR, TCK = cfg.ND, cfg.NPAIR, cfg.TCK
    SC, NSC, KC, NKC, VC, NVC = cfg.SC, cfg.NSC, cfg.KC, cfg.NKC, cfg.VC, cfg.NVC
    PW = cfg.PAIRW
    HPV = VC // 64  # heads covered per v-proj chunk

    # ---- I/O ----
    qt_hi = nc.declare_dram_parameter("qt_hi", [D, SH], BF16, isOutput=False)
    qt_lo = nc.declare_dram_parameter("qt_lo", [D, SH], BF16, isOutput=False)
    kt = nc.declare_dram_parameter("kt", [D, T], BF16, isOutput=False)
    vt = nc.declare_dram_parameter("vt", [D, T], BF16, isOutput=False)
    wq = nc.declare_dram_parameter("wq", [D, H * 64], BF16, isOutput=False)
    wk = nc.declare_dram_parameter("wk", [D, H * 64], BF16, isOutput=False)
    wv = nc.declare_dram_parameter("wv", [D, H * 64], BF16, isOutput=False)
    wo = nc.declare_dram_parameter("wo", [H * 64, D], BF16, isOutput=False)
    bq_p = nc.declare_dram_parameter("bq_p", [128, ND], F32, isOutput=False)
    bk_p = nc.declare_dram_parameter("bk_p", [128, ND], F32, isOutput=False)
    bv_r = nc.declare_dram_parameter("bv_r", [1, cfg.HV], F32, isOutput=False)
    bo_p = nc.declare_dram_parameter("bo_p", [128, ND], F32, isOutput=False)
    gamma_p = nc.declare_dram_parameter("gamma_p", [128, ND], F32, isOutput=False)
    beta_p = nc.declare_dram_parameter("beta_p", [128, ND], F32, isOutput=False)
    out = nc.declare_dram_parameter("out", [D, SH], F32, isOutput=True)

    # rearranged DRAM views: partition-chunked
    qt_hi_r = qt_hi.rearrange("(n p) s -> p n s", p=128)
    qt_lo_r = qt_lo.rearrange("(n p) s -> p n s", p=128)
    kt_r = kt.rearrange("(n p) t -> p n t", p=128)
    vt_r = vt.rearrange("(n p) t -> p n t", p=128)
    wq_r = wq.rearrange("(n p) c -> p n c", p=128)
    wk_r = wk.rearrange("(n p) c -> p n c", p=128)
    wv_r = wv.rearrange("(n p) c -> p n c", p=128)
    wo_r = wo.rearrange("(n p) c -> p n c", p=128)
    out_r = out.rearrange("(n p) s -> p n s", p=128)

    with tile.TileContext(nc) as tc, ExitStack() as ctx:
        consts = ctx.enter_context(tc.tile_pool(name="consts", bufs=1))
        wpool = ctx.enter_context(tc.tile_pool(name="wpool", bufs=1))
        streams = ctx.enter_context(tc.tile_pool(name="streams", bufs=2))
        qTp = ctx.enter_context(tc.tile_pool(name="qTp", bufs=NPAIR))
        kTp = ctx.enter_context(tc.tile_pool(name="kTp", bufs=NPAIR))
        vap = ctx.enter_context(tc.tile_pool(name="vap", bufs=TCK))
        ccp = ctx.enter_context(tc.tile_pool(name="ccp", bufs=NPAIR))
        work = ctx.enter_context(tc.tile_pool(name="work", bufs=2))
        wopool = ctx.enter_context(tc.tile_pool(name="wopool", bufs=1))
        psum = ctx.enter_context(
            tc.tile_pool(name="psum", bufs=2, space=bass.MemorySpace.PSUM))
        dram = ctx.enter_context(
            tc.tile_pool(name="dram", bufs=1, space="DRAM"))

        # ---- constants ----
        bq_sb = consts.tile([128, ND], F32)
        bk_sb = consts.tile([128, ND], F32)
        bo_sb = consts.tile([128, ND], F32)
        gamma_sb = consts.tile([128, ND], F32)
        beta_sb = consts.tile([128, ND], F32)
        bvrow = consts.tile([1, cfg.HV], F32)
        bv_bc = consts.tile([128, cfg.HV], F32)
        nc.sync.dma_start(bq_sb[:], bq_p[:])
        nc.sync.dma_start(bk_sb[:], bk_p[:])
        nc.sync.dma_start(bo_sb[:], bo_p[:])
        nc.sync.dma_start(gamma_sb[:], gamma_p[:])
        nc.sync.dma_start(beta_sb[:], beta_p[:])
        nc.sync.dma_start(bvrow[:], bv_r[:])
        nc.gpsimd.partition_broadcast(bv_bc[:], bvrow[0:1, :], channels=128)

        # ---- persistent tiles ----
        qT_tiles = [qTp.tile([128, SH], BF16, name=f"qT{p}", tag="qT")
                    for p in range(NPAIR)]
        kT_tiles = [kTp.tile([128, T], BF16, name=f"kT{p}", tag="kT")
                    for p in range(NPAIR)]
        va_tiles = [vap.tile([128, NPAIR * PW], BF16, name=f"va{t}", tag="va")
                    for t in range(TCK)]
        cc_tiles = [ccp.tile([128, SH], BF16, name=f"cc{p}", tag="cc")
                    for p in range(NPAIR)]

        # ---- v projection (+ v_aug assembly) ----
        wv_sb = wpool.tile([128, ND, H * 64], BF16, tag="w")
        nc.sync.dma_start(wv_sb[:], wv_r[:])
        for t in range(TCK):
            # init gap + ones columns of v_aug tile
            va3 = va_tiles[t].rearrange("p (q x) -> p q x", x=PW)
            nc.vector.memset(va3[:, :, 64:128], 0.0)
            nc.vector.memset(va3[:, :, 64:65], 1.0)
            vs = streams.tile([128, ND, 128], BF16, tag="instream")
            nc.sync.dma_start(vs[:], vt_r[:, :, ds(t * 128, 128)])
            for vc in range(NVC):
                ps = psum.tile([128, VC], F32, tag="mm512")
                for d in range(ND):
                    nc.tensor.matmul(ps[:], vs[:, d, :],
                                     wv_sb[:, d, ds(vc * VC, VC)],
                                     start=(d == 0), stop=(d == ND - 1))
                # scatter into v_aug A/B slots with bias add
                ps4 = ps.rearrange("p (g e v) -> p g e v", e=2, v=64)
                bv4 = bv_bc[:, ds(vc * VC, VC)].rearrange(
                    "p (g e v) -> p g e v", e=2, v=64)
                qlo = vc * (HPV // 2)
                dstA = va3[:, ds(qlo, HPV // 2), 0:64]
                dstB = va3[:, ds(qlo, HPV // 2), 128:192]
                nc.vector.tensor_tensor(
                    out=dstA, in0=ps4[:, :, 0, :], in1=bv4[:, :, 0, :], op=ALU.add)
                nc.vector.tensor_tensor(
                    out=dstB, in0=ps4[:, :, 1, :], in1=bv4[:, :, 1, :], op=ALU.add)

        if cfg.phase_limit < 2:
            dbg = work.tile([128, SH], F32, tag="sq", bufs=2)
            nc.vector.tensor_copy(dbg[:], qT_tiles[0][:, 0:SH])
            nc.sync.dma_start(out_r[:, 0, :], dbg[:])
            dbg2 = work.tile([128, SH], F32, tag="sq", bufs=2)
            nc.vector.tensor_copy(dbg2[:], kT_tiles[0][:, 0:SH])
            nc.sync.dma_start(out_r[:, 1, :], dbg2[:])
            for d in range(2, ND):
                dbg3 = work.tile([128, SH], F32, tag="sq", bufs=2)
                nc.vector.tensor_copy(dbg3[:], va_tiles[0][:, 0:SH])
                nc.sync.dma_start(out_r[:, d, :], dbg3[:])

        # ---- q projection ----
        wq_sb = wpool.tile([128, ND, H * 64], BF16, tag="w")
        nc.sync.dma_start(wq_sb[:], wq_r[:])
        for sc in range(NSC):
            qs = streams.tile([128, ND, SC], BF16, tag="instream")
            nc.sync.dma_start(qs[:], qt_hi_r[:, :, ds(sc * SC, SC)])
            for p in range(NPAIR):
                ps = psum.tile([128, SC], F32, tag="mm512")
                for d in range(ND):
                    nc.tensor.matmul(ps[:], wq_sb[:, d, ds(p * 128, 128)],
                                     qs[:, d, :], start=(d == 0), stop=(d == ND - 1))
                nc.vector.tensor_scalar(
                    out=qT_tiles[p][:, ds(sc * SC, SC)], in0=ps[:],
                    scalar1=bq_sb[:, ts(p, 1)],
                    scalar2=None, op0=ALU.add)

        # ---- per-pair k projection + attention (pipelined) ----
        wk_sb = wpool.tile([128, ND, H * 64], BF16, tag="w")
        nc.sync.dma_start(wk_sb[:], wk_r[:])
        for p in range(NPAIR if cfg.phase_limit >= 2 else 0):
            for kc in range(NKC):
                ks = streams.tile([128, ND, KC], BF16, tag="instream")
                nc.sync.dma_start(ks[:], kt_r[:, :, ds(kc * KC, KC)])
                ps = psum.tile([128, KC], F32, tag="mm512")
                for d in range(ND):
                    nc.tensor.matmul(ps[:], wk_sb[:, d, ds(p * 128, 128)],
                                     ks[:, d, :], start=(d == 0), stop=(d == ND - 1))
                nc.vector.tensor_scalar(
                    out=kT_tiles[p][:, ds(kc * KC, KC)], in0=ps[:],
                    scalar1=bk_sb[:, ts(p, 1)],
                    scalar2=None, op0=ALU.add)
            for sc in range(NSC):
                oA = psum.tile([128, SC], F32, tag="pv", name="oA")
                oB = psum.tile([128, SC], F32, tag="pv", name="oB")
                for t in range(TCK):
                    sps = psum.tile([128, 2 * SC], F32, tag="scores")
                    nc.tensor.matmul(sps[:, 0:SC],
                                     kT_tiles[p][0:64, ds(t * 128, 128)],
                                     qT_tiles[p][0:64, ds(sc * SC, SC)])
                    nc.tensor.matmul(sps[:, SC:2 * SC],
                                     kT_tiles[p][64:128, ds(t * 128, 128)],
                                     qT_tiles[p][64:128, ds(sc * SC, SC)])
                    pt = work.tile([128, 2 * SC], BF16, tag="pt", bufs=3)
                    nc.scalar.activation(pt[:], sps[:], AF.Exp, scale=cfg.scale)
                    nc.tensor.matmul(oA[:], va_tiles[t][:, ds(p * PW, 128)],
                                     pt[:, 0:SC],
                                     start=(t == 0), stop=(t == TCK - 1))
                    nc.tensor.matmul(oB[:], va_tiles[t][:, ds(p * PW + 64, 128)],
                                     pt[:, SC:2 * SC],
                                     start=(t == 0), stop=(t == TCK - 1))
                # softmax denominators: custom-DVE and partition_broadcast
                # only operate at base partition 0, so head A's den (PSUM row
                # 64) is staged to SBUF and DMA-moved to partition 0 first.
                stg = work.tile([128, SC], F32, tag="den", bufs=3)
                rcpA = work.tile([128, SC], F32, tag="den", bufs=3)
                rcpB = work.tile([128, SC], F32, tag="den", bufs=3)
                bcA = work.tile([128, SC], F32, tag="bc", bufs=2)
                bcB = work.tile([128, SC], F32, tag="bc", bufs=2)
                nc.vector.tensor_copy(stg[64:65, :], oA[64:65, :])
                nc.gpsimd.dma_start(stg[0:1, :], stg[64:65, :])
                nc.vector.reciprocal_approx_fast(out=rcpA[0:1, :], in_=stg[0:1, :])
                nc.gpsimd.partition_broadcast(bcA[0:64, :], rcpA[0:1, :], channels=64)
                nc.vector.reciprocal_approx_fast(out=rcpB[0:1, :], in_=oB[0:1, :])
                nc.gpsimd.partition_broadcast(bcB[:, :], rcpB[0:1, :], channels=128)
                nc.vector.tensor_tensor(out=cc_tiles[p][0:64, ds(sc * SC, SC)],
                                        in0=oA[0:64, :], in1=bcA[0:64, :], op=ALU.mult)
                nc.vector.tensor_tensor(out=cc_tiles[p][64:128, ds(sc * SC, SC)],
                                        in0=oB[64:128, :], in1=bcB[64:128, :], op=ALU.mult)

        if cfg.phase_limit == 2:
            for d in range(ND):
                dbg = work.tile([128, SH], F32, tag="sq", bufs=2)
                nc.vector.tensor_copy(dbg[:], cc_tiles[d % NPAIR][:, 0:SH])
                nc.sync.dma_start(out_r[:, d, :], dbg[:])
        if cfg.phase_limit < 3:
            nc.finish_tc = True  # marker; skip rest
        # ---- out projection + residual + BN stats ----
        if cfg.phase_limit < 3:
            ND_ = 0
        else:
            ND_ = ND
        wo_sb = wopool.tile([128, ND, D], BF16, tag="wo")
        nc.sync.dma_start(wo_sb[:], wo_r[:])
        outT_d = dram.tile([128, ND, SH], F32)
        stats = consts.tile([128, 2 * ND], F32)
        sqscratch = work.tile([128, SH], F32, tag="sq", bufs=2)
        for d in range(ND_):
            ot = work.tile([128, SH], F32, tag="ot", bufs=2)
            for sc in range(NSC):
                qh = streams.tile([128, SC], BF16, tag="qh")
                ql = streams.tile([128, SC], BF16, tag="ql")
                nc.sync.dma_start(qh[:], qt_hi_r[:, d, ds(sc * SC, SC)])
                nc.sync.dma_start(ql[:], qt_lo_r[:, d, ds(sc * SC, SC)])
                ps = psum.tile([128, SC], F32, tag="mm512")
                for p in range(NPAIR):
                    nc.tensor.matmul(ps[:], wo_sb[:, p, ds(d * 128, 128)],
                                     cc_tiles[p][:, ds(sc * SC, SC)],
                                     start=(p == 0), stop=(p == NPAIR - 1))
                seg = ot[:, ds(sc * SC, SC)]
                nc.vector.tensor_scalar(out=seg, in0=ps[:],
                                        scalar1=bo_sb[:, ts(d, 1)],
                                        scalar2=None, op0=ALU.add)
                nc.vector.tensor_tensor(out=seg, in0=seg, in1=qh[:], op=ALU.add)
                nc.vector.tensor_tensor(out=seg, in0=seg, in1=ql[:], op=ALU.add)
            nc.vector.tensor_reduce(out=stats[:, ts(d, 1)], in_=ot[:],
                                    axis=mybir.AxisListType.X, op=ALU.add)
            nc.scalar.activation(sqscratch[:], ot[:], AF.Square,
                                 accum_out=stats[:, ts(ND + d, 1)])
            nc.sync.dma_start(outT_d[:, d, :], ot[:])

        if cfg.phase_limit >= 3:
            # ---- BN stats allreduce ----
            st_in = dram.tile([128, 2 * ND], F32)
            st_out = dram.tile([128, 2 * ND], F32)
            nc.sync.dma_start(st_in[:], stats[:])
            if cfg.use_collective:
                nc.gpsimd.collective_compute(
                    "AllReduce", ALU.add,
                    replica_groups=[list(range(cfg.n_cores))],
                    ins=[st_in.opt()], outs=[st_out.opt()])
            else:
                nc.sync.dma_start(st_out[:], st_in[:])
            gstats = consts.tile([128, 2 * ND], F32)
            nc.sync.dma_start(gstats[:], st_out[:])

            # ---- BN scale/shift ----
            inv_n = 1.0 / float(cfg.n_total)
            mean = consts.tile([128, ND], F32)
            ex2 = consts.tile([128, ND], F32)
            var = consts.tile([128, ND], F32)
            std = consts.tile([128, ND], F32)
            rstd = consts.tile([128, ND], F32)
            scale_t = consts.tile([128, ND], F32)
            shift_t = consts.tile([128, ND], F32)
            nc.vector.tensor_scalar(out=mean[:], in0=gstats[:, 0:ND],
                                    scalar1=inv_n, scalar2=None, op0=ALU.mult)
            nc.vector.tensor_scalar(out=ex2[:], in0=gstats[:, ds(ND, ND)],
                                    scalar1=inv_n, scalar2=None, op0=ALU.mult)
            nc.vector.tensor_tensor(out=var[:], in0=mean[:], in1=mean[:], op=ALU.mult)
            nc.vector.tensor_tensor(out=var[:], in0=ex2[:], in1=var[:], op=ALU.subtract)
            nc.vector.tensor_scalar(out=var[:], in0=var[:], scalar1=cfg.eps,
                                    scalar2=None, op0=ALU.add)
            nc.scalar.activation(std[:], var[:], AF.Sqrt)
            nc.vector.reciprocal(rstd[:], std[:])
            nc.vector.tensor_tensor(out=scale_t[:], in0=rstd[:], in1=gamma_sb[:],
                                    op=ALU.mult)
            nc.vector.tensor_tensor(out=shift_t[:], in0=mean[:], in1=scale_t[:],
                                    op=ALU.mult)
            nc.vector.tensor_tensor(out=shift_t[:], in0=beta_sb[:], in1=shift_t[:],
                                    op=ALU.subtract)

            # ---- BN apply + output ----
            for d in range(ND):
                otb = work.tile([128, SH], F32, tag="ot", bufs=2)
                nc.sync.dma_start(otb[:], outT_d[:, d, :])
                fin = work.tile([128, SH], F32, tag="sq", bufs=2)
                nc.vector.tensor_scalar(out=fin[:], in0=otb[:],
                                        scalar1=scale_t[:, ts(d, 1)],
                                        scalar2=shift_t[:, ts(d, 1)],
                                        op0=ALU.mult, op1=ALU.add)
                nc.sync.dma_start(out_r[:, d, :], fin[:])

    nc.compile()
    return nc


def prep_core_inputs(cfg, Q, K, V, Wq, bq, Wk, bk, Wv, bv, Wo, bo, gamma, beta,
                     b, half):
    """Build the in_map for core (b, half). Inputs are numpy fp32."""
    D, H, SH = cfg.D, cfg.H, cfg.S_HALF
    ND = cfg.ND
    s0 = half * SH
    qt = np.ascontiguousarray(Q[b, s0:s0 + SH, :].T)      # [D, SH]
    qt_hi = qt.astype(BF)
    qt_lo = (qt - qt_hi.astype(np.float32)).astype(BF)
    kt = np.ascontiguousarray(K[b].T).astype(BF)          # [D, T]
    vt = np.ascontiguousarray(V[b].T).astype(BF)
    wq = Wq.transpose(1, 0, 2).reshape(D, H * 64).astype(BF)
    wk = Wk.transpose(1, 0, 2).reshape(D, H * 64).astype(BF)
    wv = Wv.transpose(1, 0, 2).reshape(D, H * 64).astype(BF)
    wo = np.asarray(Wo, np.float32).astype(BF)            # [H*64, D]
    pack = lambda v: np.ascontiguousarray(
        np.asarray(v, np.float32).reshape(ND, 128).T)
    return {
        "qt_hi": qt_hi, "qt_lo": qt_lo, "kt": kt, "vt": vt,
        "wq": wq, "wk": wk, "wv": wv, "wo": wo,
        "bq_p": pack(bq), "bk_p": pack(bk),
        "bv_r": np.asarray(bv, np.float32).reshape(1, H * 64).copy(),
        "bo_p": pack(bo), "gamma_p": pack(gamma), "beta_p": pack(beta),
    }


_PROGRAM_CACHE = {}


def _get_program(cfg):
    key = (cfg.D, cfg.H, cfg.S_HALF, cfg.T, cfg.n_cores)
    if key not in _PROGRAM_CACHE:
        _PROGRAM_CACHE[key] = build_program(cfg)
    return _PROGRAM_CACHE[key]


def run(inputs, trace=False, trace_kwargs=None):
    """Run the SPMD kernel; returns (output [B,D,S] fp32, BassKernelResults)."""
    cfg = Cfg()
    args = [np.asarray(inputs[k], np.float32) for k in
            ("Q", "K", "V", "Wq", "bq", "Wk", "bk", "Wv", "bv", "Wo", "bo",
             "gamma", "beta")]
    in_maps = [prep_core_inputs(cfg, *args, i // 2, i % 2)
               for i in range(cfg.n_cores)]
    nc = _get_program(cfg)
    res = run_bass_kernel_spmd(nc, in_maps, list(range(cfg.n_cores)),
                               trace=trace, trace_kwargs=trace_kwargs or {})
    B = inputs["Q"].shape[0]
    S = inputs["Q"].shape[1]
    outp = np.empty((B, cfg.D, S), np.float32)
    for i in range(cfg.n_cores):
        b, half = i // 2, i % 2
        outp[b, :, half * cfg.S_HALF:(half + 1) * cfg.S_HALF] = \
            res.results[i]["out"]
    return outp, res


def kernel(**inputs) -> np.ndarray:
    out, _ = run(inputs, trace=False)
    return out



# revision 5
# speedup vs baseline: 1.1987x; 1.1987x over previous
# Trainium2 Bass SPMD kernel for nn_MultiHeadAttn_16492674416882.
#
# kernel(**inputs) takes the FULL fp32 inputs and returns the FULL
# (B, D, S) output, running a fused per-core program on 8 NeuronCores.
#
# Sharding: core i handles batch b=i//2 and query-half h=i%2 (1024 of the
# 2048 positions). K/V projections for a batch are computed by both cores of
# the pair (cheap duplication), which removes every large collective; the
# only cross-core communication is an 8KB AllReduce of BatchNorm statistics.
#
# v2: all projections and the attention*V matmuls run as fp8e4 (E4M3)
# DoubleRow matmuls (2 contraction rows per partition, 2x PE throughput).
# Weights are pre-scaled by 32 on the host so their ~0.02-sigma values sit in
# e4m3's normal range; V output is kept scaled by 32 (va = 32*(v+bv)) so the
# attention output (sigma ~0.01) stays normal in fp8 as well. The score
# matmuls stay bf16 (they are output-rate-bound; fp8 wouldn't help), with
# softmax exp running on the Activation engine (true exp, fp8 out) for most
# tiles and on the DVE for a configurable fraction via the Schraudolph
# bit-trick (x*8/ln2 + 55.68 rounded to int8 == e4m3 bits of exp(x)), which
# balances the two engines. Softmax denominators come from ones-stationary
# DoubleRow matmuls accumulated alongside the attention values in the same
# PSUM tile. The residual is added exactly in fp32 (qt32 = Q^T + bo), the
# pre-BN output stays SBUF-resident, and BatchNorm statistics are
# all-reduced (8KB) before the final scale/shift.

import math
import os
import sys
from contextlib import ExitStack
from dataclasses import dataclass

import numpy as np
import ml_dtypes

for _p in ("/root/.axon_site/_ro/trn_rl_repo", "/opt/trn_rl_repo"):
    if _p not in sys.path and os.path.isdir(_p):
        sys.path.append(_p)

import concourse.bass as bass
import concourse.tile as tile
from concourse import bacc, mybir
from concourse.bass import ds, ts
from concourse.bass_utils import run_bass_kernel_spmd

F32 = mybir.dt.float32
BF16 = mybir.dt.bfloat16
F8E4 = mybir.dt.float8e4
I8 = mybir.dt.int8
AF = mybir.ActivationFunctionType
ALU = mybir.AluOpType
PM = mybir.MatmulPerfMode
BF = ml_dtypes.bfloat16
F8 = ml_dtypes.float8_e4m3


@dataclass
class Cfg:
    D: int = 1024          # model dim (== H*64)
    H: int = 16            # heads
    SH: int = 1024         # queries per core
    T: int = 2048          # kv length
    n_cores: int = 8
    n_total: int = 8192    # BN normalization count (B*S)
    use_collective: bool = True
    eps: float = 1e-5
    scale: float = 1.0 / 32.0    # sqrt(1/1024), exactly 1/32
    wscale: float = 32.0         # fp8 weight prescale
    exp_dve_mod: int = 4         # ci % mod == mod-1 -> exp on DVE bit-trick

    @property
    def ND(self): return self.D // 128
    @property
    def NPAIR(self): return self.H // 2
    @property
    def TCK(self): return self.T // 128     # 128-t chunks
    @property
    def NC2(self): return self.T // 256     # 256-t chunks
    @property
    def HV(self): return self.H * 64


def build_program(cfg: Cfg) -> bass.Bass:
    nc = bacc.Bacc("TRN2", target_bir_lowering=False, debug=False,
                   num_devices=cfg.n_cores)
    D, H, SH, T = cfg.D, cfg.H, cfg.SH, cfg.T
    ND, NPAIR, TCK, NC2 = cfg.ND, cfg.NPAIR, cfg.TCK, cfg.NC2
    HV = cfg.HV
    INV_W = 1.0 / cfg.wscale
    INV_WSQ = 1.0 / (cfg.wscale * cfg.wscale)
    # fast-exp (Schraudolph) constants for e4m3 bits, round-to-nearest on HW
    FE_A = cfg.scale * 8.0 / math.log(2.0)
    FE_B = 56.0 - 0.344

    # ---- I/O ----
    qt8 = nc.declare_dram_parameter("qt8", [D, SH], F8E4, isOutput=False)
    qt32 = nc.declare_dram_parameter("qt32", [D, SH], F32, isOutput=False)
    kt8 = nc.declare_dram_parameter("kt8", [D, T], F8E4, isOutput=False)
    vt8 = nc.declare_dram_parameter("vt8", [D, T], F8E4, isOutput=False)
    wq8 = nc.declare_dram_parameter("wq8", [D, HV], F8E4, isOutput=False)
    wk8 = nc.declare_dram_parameter("wk8", [D, HV], F8E4, isOutput=False)
    wv8 = nc.declare_dram_parameter("wv8", [D, HV], F8E4, isOutput=False)
    wo8 = nc.declare_dram_parameter("wo8", [HV, D], F8E4, isOutput=False)
    bq_p = nc.declare_dram_parameter("bq_p", [128, ND], F32, isOutput=False)
    bk_p = nc.declare_dram_parameter("bk_p", [128, ND], F32, isOutput=False)
    bv_r = nc.declare_dram_parameter("bv_r", [1, HV], F32, isOutput=False)
    gamma_p = nc.declare_dram_parameter("gamma_p", [128, ND], F32, isOutput=False)
    beta_p = nc.declare_dram_parameter("beta_p", [128, ND], F32, isOutput=False)
    out = nc.declare_dram_parameter("out", [D, SH], F32, isOutput=True)

    qt8_r = qt8.rearrange("(n p) s -> p n s", p=128)
    qt32_r = qt32.rearrange("(n p) s -> p n s", p=128)
    kt8_r = kt8.rearrange("(n p) t -> p n t", p=128)
    vt8_r = vt8.rearrange("(n p) t -> p n t", p=128)
    wq_r = wq8.rearrange("(n p) c -> p n c", p=128)
    wk_r = wk8.rearrange("(n p) c -> p n c", p=128)
    wv_r = wv8.rearrange("(n p) c -> p n c", p=128)
    wo_r = wo8.rearrange("(n p) c -> p n c", p=128)
    out_r = out.rearrange("(n p) s -> p n s", p=128)

    with tile.TileContext(nc) as tc, ExitStack() as ctx:
        consts = ctx.enter_context(tc.tile_pool(name="consts", bufs=1))
        wpool = ctx.enter_context(tc.tile_pool(name="wpool", bufs=1))
        bigp = ctx.enter_context(tc.tile_pool(name="bigp", bufs=1))
        streams = ctx.enter_context(tc.tile_pool(name="streams", bufs=2))
        work = ctx.enter_context(tc.tile_pool(name="work", bufs=2))
        psum = ctx.enter_context(
            tc.tile_pool(name="psum", bufs=2, space=bass.MemorySpace.PSUM))
        dram = ctx.enter_context(
            tc.tile_pool(name="dram", bufs=1, space="DRAM"))

        # ---- constants ----
        bq_sb = consts.tile([128, ND], F32)
        bk_sb = consts.tile([128, ND], F32)
        gamma_sb = consts.tile([128, ND], F32)
        beta_sb = consts.tile([128, ND], F32)
        bvrow = consts.tile([1, HV], F32)
        bv_bc = consts.tile([128, HV], F32)
        ones8 = consts.tile([128, 2, 64], F8E4)
        stats_p = consts.tile([128, 4 * ND], F32)  # per-sc partials
        stats = consts.tile([128, 2 * ND], F32)
        sqscr = consts.tile([128, 512], F32)
        nc.sync.dma_start(bq_sb[:], bq_p[:])
        nc.sync.dma_start(bk_sb[:], bk_p[:])
        nc.sync.dma_start(gamma_sb[:], gamma_p[:])
        nc.sync.dma_start(beta_sb[:], beta_p[:])
        nc.sync.dma_start(bvrow[:], bv_r[:])
        nc.gpsimd.partition_broadcast(bv_bc[:], bvrow[0:1, :], channels=128)
        nc.vector.memset(ones8[:], 1.0)

        # ---- weights + resident inputs ----
        wk_sb = wpool.tile([128, ND, HV], F8E4, tag="wk")
        wv_sb = wpool.tile([128, ND, HV], F8E4, tag="wv")
        wo_sb = wpool.tile([128, ND, D], F8E4, tag="wo")
        qt8_sb = wpool.tile([128, ND, SH], F8E4, tag="qt8")
        kt8_sb = wpool.tile([128, ND, T], F8E4, tag="kt8")
        # wq shares a buffer with P (q projection fully precedes attention)
        wq_sb = bigp.tile([128, ND, HV], F8E4, tag="wqP", name="wq_sb")
        nc.sync.dma_start(wk_sb[:], wk_r[:])
        nc.sync.dma_start(wq_sb[:], wq_r[:])
        nc.sync.dma_start(wv_sb[:], wv_r[:])
        nc.sync.dma_start(wo_sb[:], wo_r[:])
        nc.sync.dma_start(qt8_sb[:], qt8_r[:])
        nc.sync.dma_start(kt8_sb[:], kt8_r[:])

        # ---- persistent tiles ----
        qT = bigp.tile([128, NPAIR, SH], BF16, tag="qT")
        kT = bigp.tile([128, NPAIR, T], BF16, tag="kT")
        va = bigp.tile([128, TCK, HV], F8E4, tag="va")
        cc = bigp.tile([128, NPAIR, SH], F8E4, tag="cc")
        ot = bigp.tile([128, ND, SH], F32, tag="ot")

        def kproj(j):
            for tc_ in range(4):
                ps = psum.tile([128, 1024], F32, tag="pp")
                for h in range(2):
                    for u in range(4):
                        nc.tensor.matmul(
                            ps[ds(h * 64, 64), 0:512],
                            wk_sb[:, ds(2 * u, 2), ds(j * 128 + h * 64, 64)],
                            kt8_sb[:, ds(2 * u, 2), ds(tc_ * 512, 512)],
                            start=(u == 0), stop=(u == 3), perf_mode=PM.DoubleRow)
                nc.vector.tensor_scalar(
                    out=kT[:, j, ds(tc_ * 512, 512)], in0=ps[:, 0:512],
                    scalar1=INV_W, scalar2=bk_sb[:, ts(j, 1)],
                    op0=ALU.mult, op1=ALU.add)

        def qproj(j):
            ps = psum.tile([128, 1024], F32, tag="pp")
            for h in range(2):
                for sc in range(2):
                    for u in range(4):
                        nc.tensor.matmul(
                            ps[ds(h * 64, 64), ds(sc * 512, 512)],
                            wq_sb[:, ds(2 * u, 2), ds(j * 128 + h * 64, 64)],
                            qt8_sb[:, ds(2 * u, 2), ds(sc * 512, 512)],
                            start=(u == 0), stop=(u == 3), perf_mode=PM.DoubleRow)
            nc.scalar.activation(qT[:, j, :], ps[:], AF.Identity,
                                 bias=bq_sb[:, ts(j, 1)], scale=INV_W)

        def vproj(c):
            vs = streams.tile([128, ND, 128], F8E4, tag="vs")
            nc.sync.dma_start(vs[:], vt8_r[:, :, ds(c * 128, 128)])
            ps = psum.tile([128, 1024], F32, tag="pp")
            for h in range(2):
                for w in range(2):
                    for u in range(4):
                        nc.tensor.matmul(
                            ps[ds(h * 64, 64), ds(w * 512, 512)],
                            vs[:, ds(2 * u, 2), ds(h * 64, 64)],
                            wv_sb[:, ds(2 * u, 2), ds(w * 512, 512)],
                            start=(u == 0), stop=(u == 3), perf_mode=PM.DoubleRow)
            # va = psum + 32*bv  (psum is 32*v since wv is prescaled)
            nc.vector.tensor_tensor(out=va[:, c, :], in0=ps[:], in1=bv_bc[:],
                                    op=ALU.add)

        P_all = bigp.tile([128, TCK, SH], F8E4, tag="wqP", name="P_all")

        def attn(j, sc, do_vproj):
            vd = psum.tile([128, 1024], F32, tag="vd")
            for c2 in range(NC2):
                if do_vproj:
                    for c in (2 * c2 + 2, 2 * c2 + 3):
                        if c < TCK:
                            vproj(c)
                for ci in (2 * c2, 2 * c2 + 1):
                    ps = psum.tile([128, 1024], F32, tag="pp")
                    nc.tensor.matmul(ps[:, 0:512],
                                     kT[0:64, j, ds(ci * 128, 128)],
                                     qT[0:64, j, ds(sc * 512, 512)])
                    nc.tensor.matmul(ps[:, 512:1024],
                                     kT[64:128, j, ds(ci * 128, 128)],
                                     qT[64:128, j, ds(sc * 512, 512)])
                    m = cfg.exp_dve_mod
                    if m > 0 and ci % m == m - 1:
                        nc.vector.tensor_scalar(
                            out=P_all[:, ci, :].bitcast(I8), in0=ps[:],
                            scalar1=FE_A, scalar2=FE_B,
                            op0=ALU.mult, op1=ALU.add)
                    else:
                        nc.scalar.activation(P_all[:, ci, :], ps[:], AF.Exp,
                                             scale=cfg.scale)
                st, sp = (c2 == 0), (c2 == NC2 - 1)
                nc.tensor.matmul(vd[0:64, 0:512],
                                 va[:, ds(2 * c2, 2), ds(j * 128, 64)],
                                 P_all[:, ds(2 * c2, 2), 0:512],
                                 start=st, stop=sp, perf_mode=PM.DoubleRow)
                nc.tensor.matmul(vd[64:128, 0:512],
                                 va[:, ds(2 * c2, 2), ds(j * 128 + 64, 64)],
                                 P_all[:, ds(2 * c2, 2), 512:1024],
                                 start=st, stop=sp, perf_mode=PM.DoubleRow)
                nc.tensor.matmul(vd[0:1, 512:1024], ones8[:, :, 0:1],
                                 P_all[:, ds(2 * c2, 2), 0:512],
                                 start=st, stop=sp, perf_mode=PM.DoubleRow)
                nc.tensor.matmul(vd[64:65, 512:1024], ones8[:, :, 0:1],
                                 P_all[:, ds(2 * c2, 2), 512:1024],
                                 start=st, stop=sp, perf_mode=PM.DoubleRow)
            # normalize: cc = vals * (1/den)   (cc ends up as 32*attn_out)
            stg = work.tile([128, 512], F32, tag="stg")
            rcp = work.tile([1, 1024], F32, tag="rcp")
            bc = work.tile([128, 1024], F32, tag="bc")
            nc.scalar.activation(stg[64:65, 0:512], vd[64:65, 512:1024], AF.Copy)
            nc.gpsimd.dma_start(stg[0:1, 0:512], stg[64:65, 0:512])
            nc.vector.reciprocal_approx_fast(out=rcp[0:1, 0:512],
                                             in_=vd[0:1, 512:1024])
            nc.vector.reciprocal_approx_fast(out=rcp[0:1, 512:1024],
                                             in_=stg[0:1, 0:512])
            nc.gpsimd.partition_broadcast(bc[:], rcp[0:1, :], channels=128)
            nc.vector.tensor_tensor(out=cc[0:64, j, ds(sc * 512, 512)],
                                    in0=vd[0:64, 0:512], in1=bc[0:64, 0:512],
                                    op=ALU.mult)
            nc.vector.tensor_tensor(out=cc[64:128, j, ds(sc * 512, 512)],
                                    in0=vd[64:128, 0:512],
                                    in1=bc[64:128, 512:1024], op=ALU.mult)

        def oproj(sc):
            for d in range(ND):
                ps = psum.tile([128, 1024], F32, tag="pp")
                for h in range(2):
                    for u in range(4):
                        nc.tensor.matmul(
                            ps[ds(h * 64, 64), 0:512],
                            wo_sb[:, ds(2 * u, 2), ds(d * 128 + h * 64, 64)],
                            cc[:, ds(2 * u, 2), ds(sc * 512, 512)],
                            start=(u == 0), stop=(u == 3), perf_mode=PM.DoubleRow)
                qres = streams.tile([128, 512], F32, tag="qres")
                nc.sync.dma_start(qres[:], qt32_r[:, d, ds(sc * 512, 512)])
                seg = ot[:, d, ds(sc * 512, 512)]
                nc.vector.scalar_tensor_tensor(
                    out=seg, in0=ps[:, 0:512], scalar=INV_WSQ, in1=qres[:],
                    op0=ALU.mult, op1=ALU.add)
                nc.vector.tensor_reduce(out=stats_p[:, ts(sc * 2 * ND + d, 1)],
                                        in_=seg, axis=mybir.AxisListType.X,
                                        op=ALU.add)
                nc.scalar.activation(sqscr[:], seg, AF.Square,
                                     accum_out=stats_p[:, ts(sc * 2 * ND + ND + d, 1)])

        # ---- emission ----
        kproj(0)
        for j in range(NPAIR):
            qproj(j)
        vproj(0)
        vproj(1)
        for sc in range(2):
            for j in range(NPAIR):
                if sc == 0 and j > 0:
                    kproj(j)
                attn(j, sc, do_vproj=(sc == 0 and j == 0))
            oproj(sc)

        # combine per-sc stat partials
        nc.vector.tensor_tensor(out=stats[:], in0=stats_p[:, 0:2 * ND],
                                in1=stats_p[:, ds(2 * ND, 2 * ND)], op=ALU.add)

        # ---- BN stats allreduce ----
        st_in = dram.tile([128, 2 * ND], F32)
        st_out = dram.tile([128, 2 * ND], F32)
        nc.sync.dma_start(st_in[:], stats[:])
        if cfg.use_collective:
            nc.gpsimd.collective_compute(
                "AllReduce", ALU.add,
                replica_groups=[list(range(cfg.n_cores))],
                ins=[st_in.opt()], outs=[st_out.opt()])
        else:
            nc.sync.dma_start(st_out[:], st_in[:])
        gstats = consts.tile([128, 2 * ND], F32)
        nc.sync.dma_start(gstats[:], st_out[:])

        # ---- BN scale/shift ----
        inv_n = 1.0 / float(cfg.n_total)
        mean = consts.tile([128, ND], F32)
        ex2 = consts.tile([128, ND], F32)
        var = consts.tile([128, ND], F32)
        std = consts.tile([128, ND], F32)
        rstd = consts.tile([128, ND], F32)
        scale_t = consts.tile([128, ND], F32)
        shift_t = consts.tile([128, ND], F32)
        nc.vector.tensor_scalar(out=mean[:], in0=gstats[:, 0:ND],
                                scalar1=inv_n, scalar2=None, op0=ALU.mult)
        nc.vector.tensor_scalar(out=ex2[:], in0=gstats[:, ds(ND, ND)],
                                scalar1=inv_n, scalar2=None, op0=ALU.mult)
        nc.vector.tensor_tensor(out=var[:], in0=mean[:], in1=mean[:], op=ALU.mult)
        nc.vector.tensor_tensor(out=var[:], in0=ex2[:], in1=var[:], op=ALU.subtract)
        nc.vector.tensor_scalar(out=var[:], in0=var[:], scalar1=cfg.eps,
                                scalar2=None, op0=ALU.add)
        nc.scalar.activation(std[:], var[:], AF.Sqrt)
        nc.vector.reciprocal(rstd[:], std[:])
        nc.vector.tensor_tensor(out=scale_t[:], in0=rstd[:], in1=gamma_sb[:],
                                op=ALU.mult)
        nc.vector.tensor_tensor(out=shift_t[:], in0=mean[:], in1=scale_t[:],
                                op=ALU.mult)
        nc.vector.tensor_tensor(out=shift_t[:], in0=beta_sb[:], in1=shift_t[:],
                                op=ALU.subtract)

        # ---- BN apply + output (split across engines) ----
        for d in range(ND):
            fin = work.tile([128, 1024], F32, tag="bc")
            eng = (nc.vector, nc.gpsimd, nc.vector)[d % 3]
            eng.tensor_scalar(out=fin[:], in0=ot[:, d, :],
                              scalar1=scale_t[:, ts(d, 1)],
                              scalar2=shift_t[:, ts(d, 1)],
                              op0=ALU.mult, op1=ALU.add)
            nc.sync.dma_start(out_r[:, d, :], fin[:])

    nc.compile()
    return nc


def prep_core_inputs(cfg, Q, K, V, Wq, bq, Wk, bk, Wv, bv, Wo, bo, gamma, beta,
                     b, half, shared):
    """Build the in_map for core (b, half). Inputs are numpy fp32."""
    D, H, SH = cfg.D, cfg.H, cfg.SH
    ND = cfg.ND
    s0 = half * SH
    key = ("kv", b)
    if key not in shared:
        kt = np.ascontiguousarray(K[b].T)
        vt = np.ascontiguousarray(V[b].T)
        shared[key] = (kt.astype(F8), vt.astype(F8))
    kt8, vt8 = shared[key]
    qt = np.ascontiguousarray(Q[b, s0:s0 + SH, :].T)      # [D, SH]
    return {
        "qt8": qt.astype(F8),
        "qt32": qt + np.asarray(bo, np.float32)[:, None],
        "kt8": kt8, "vt8": vt8,
        "wq8": shared["wq8"], "wk8": shared["wk8"], "wv8": shared["wv8"],
        "wo8": shared["wo8"],
        "bq_p": shared["bq_p"], "bk_p": shared["bk_p"],
        "bv_r": shared["bv_r"],
        "gamma_p": shared["gamma_p"], "beta_p": shared["beta_p"],
    }


_PROGRAM_CACHE = {}


def _get_program(cfg):
    key = (cfg.D, cfg.H, cfg.SH, cfg.T, cfg.n_cores, cfg.exp_dve_mod)
    if key not in _PROGRAM_CACHE:
        _PROGRAM_CACHE[key] = build_program(cfg)
    return _PROGRAM_CACHE[key]


def run(inputs, trace=False, trace_kwargs=None):
    """Run the SPMD kernel; returns (output [B,D,S] fp32, BassKernelResults)."""
    cfg = Cfg()
    args = [np.asarray(inputs[k], np.float32) for k in
            ("Q", "K", "V", "Wq", "bq", "Wk", "bk", "Wv", "bv", "Wo", "bo",
             "gamma", "beta")]
    Q, K, V, Wq, bq, Wk, bk, Wv, bv, Wo, bo, gamma, beta = args
    D, H, ND, ws = cfg.D, cfg.H, cfg.ND, cfg.wscale
    pack = lambda v: np.ascontiguousarray(
        np.asarray(v, np.float32).reshape(ND, 128).T)
    shared = {
        "wq8": (Wq.transpose(1, 0, 2).reshape(D, H * 64) * ws).astype(F8),
        "wk8": (Wk.transpose(1, 0, 2).reshape(D, H * 64) * ws).astype(F8),
        "wv8": (Wv.transpose(1, 0, 2).reshape(D, H * 64) * ws).astype(F8),
        "wo8": (np.asarray(Wo, np.float32) * ws).astype(F8),
        "bq_p": pack(bq), "bk_p": pack(bk),
        "bv_r": (np.asarray(bv, np.float32).reshape(1, H * 64) * ws).copy(),
        "gamma_p": pack(gamma), "beta_p": pack(beta),
    }
    in_maps = [prep_core_inputs(cfg, *args, i // 2, i % 2, shared)
               for i in range(cfg.n_cores)]
    nc = _get_program(cfg)
    res = run_bass_kernel_spmd(nc, in_maps, list(range(cfg.n_cores)),
                               trace=trace, trace_kwargs=trace_kwargs or {})
    B = inputs["Q"].shape[0]
    S = inputs["Q"].shape[1]
    outp = np.empty((B, cfg.D, S), np.float32)
    for i in range(cfg.n_cores):
        b, half = i // 2, i % 2
        outp[b, :, half * cfg.SH:(half + 1) * cfg.SH] = res.results[i]["out"]
    return outp, res


def kernel(**inputs) -> np.ndarray:
    out, _ = run(inputs, trace=False)
    return out


# revision 10
# speedup vs baseline: 1.4744x; 1.2300x over previous
# Trainium2 Bass SPMD kernel for nn_MultiHeadAttn_16492674416882.
#
# kernel(**inputs) takes the FULL fp32 inputs and returns the FULL
# (B, D, S) output, running a fused per-core program on 8 NeuronCores.
#
# Sharding: core i handles batch b=i//2 and query-half h=i%2 (1024 of the
# 2048 positions). K/V projections for a batch are computed by both cores of
# the pair (cheap duplication), which removes every large collective; the
# only cross-core communication is an 8KB AllReduce of BatchNorm statistics.
#
# v2: all projections and the attention*V matmuls run as fp8e4 (E4M3)
# DoubleRow matmuls (2 contraction rows per partition, 2x PE throughput).
# Weights are pre-scaled by 32 on the host so their ~0.02-sigma values sit in
# e4m3's normal range; V output is kept scaled by 32 (va = 32*(v+bv)) so the
# attention output (sigma ~0.01) stays normal in fp8 as well. The score
# matmuls stay bf16 (they are output-rate-bound; fp8 wouldn't help), with
# softmax exp running on the Activation engine (true exp, fp8 out) for most
# tiles and on the DVE for a configurable fraction via the Schraudolph
# bit-trick (x*8/ln2 + 55.68 rounded to int8 == e4m3 bits of exp(x)), which
# balances the two engines. Softmax denominators come from ones-stationary
# DoubleRow matmuls accumulated alongside the attention values in the same
# PSUM tile. The residual is added exactly in fp32 (qt32 = Q^T + bo), the
# pre-BN output stays SBUF-resident, and BatchNorm statistics are
# all-reduced (8KB) before the final scale/shift.

import math
import os
import sys
from contextlib import ExitStack
from dataclasses import dataclass

import numpy as np
import ml_dtypes

for _p in ("/root/.axon_site/_ro/trn_rl_repo", "/opt/trn_rl_repo"):
    if _p not in sys.path and os.path.isdir(_p):
        sys.path.append(_p)

import concourse.bass as bass
import concourse.tile as tile
from concourse import bacc, mybir
from concourse.bass import ds, ts
from concourse.bass_utils import run_bass_kernel_spmd

F32 = mybir.dt.float32
BF16 = mybir.dt.bfloat16
F8E4 = mybir.dt.float8e4
I8 = mybir.dt.int8
AF = mybir.ActivationFunctionType
ALU = mybir.AluOpType
PM = mybir.MatmulPerfMode
BF = ml_dtypes.bfloat16
F8 = ml_dtypes.float8_e4m3


@dataclass
class Cfg:
    D: int = 1024          # model dim (== H*64)
    H: int = 16            # heads
    SH: int = 1024         # queries per core
    T: int = 2048          # kv length
    n_cores: int = 8
    n_total: int = 8192    # BN normalization count (B*S)
    use_collective: bool = True
    eps: float = 1e-5
    scale: float = 1.0 / 32.0    # sqrt(1/1024), exactly 1/32
    wscale: float = 32.0         # fp8 weight prescale
    exp_dve_mod: int = 4         # ci % mod == mod-1 -> exp on DVE bit-trick

    exp_pat: str = "ADADADADADADADAD"  # exp engine per ci%16: A=Act, D=DVE
    pump_per_slot: int = 1       # feeder thunks emitted per c2 slot

    @property
    def ND(self): return self.D // 128
    @property
    def NPAIR(self): return self.H // 2
    @property
    def TCK(self): return self.T // 128     # 128-t chunks
    @property
    def NC2(self): return self.T // 256     # 256-t chunks
    @property
    def HV(self): return self.H * 64


def build_program(cfg: Cfg) -> bass.Bass:
    nc = bacc.Bacc("TRN2", target_bir_lowering=False, debug=False,
                   num_devices=cfg.n_cores)
    D, H, SH, T = cfg.D, cfg.H, cfg.SH, cfg.T
    ND, NPAIR, TCK, NC2 = cfg.ND, cfg.NPAIR, cfg.TCK, cfg.NC2
    HV = cfg.HV
    INV_W = 1.0 / cfg.wscale
    INV_WSQ = 1.0 / (cfg.wscale * cfg.wscale)
    # fast-exp (Schraudolph) constants for e4m3 bits, round-to-nearest on HW
    FE_A = cfg.scale * 8.0 / math.log(2.0)
    FE_B = 56.0 - 0.344

    # ---- I/O ----
    qt8 = nc.declare_dram_parameter("qt8", [D, SH], F8E4, isOutput=False)
    qt32 = nc.declare_dram_parameter("qt32", [D, SH], F32, isOutput=False)
    kt8 = nc.declare_dram_parameter("kt8", [D, T], F8E4, isOutput=False)
    vt8 = nc.declare_dram_parameter("vt8", [D, T], F8E4, isOutput=False)
    wq8 = nc.declare_dram_parameter("wq8", [D, HV], F8E4, isOutput=False)
    wk8 = nc.declare_dram_parameter("wk8", [D, HV], F8E4, isOutput=False)
    wv8 = nc.declare_dram_parameter("wv8", [D, HV], F8E4, isOutput=False)
    wo8 = nc.declare_dram_parameter("wo8", [HV, D], F8E4, isOutput=False)
    bq_p = nc.declare_dram_parameter("bq_p", [128, ND], F32, isOutput=False)
    bk_p = nc.declare_dram_parameter("bk_p", [128, ND], F32, isOutput=False)
    bv_r = nc.declare_dram_parameter("bv_r", [1, HV], F32, isOutput=False)
    gamma_p = nc.declare_dram_parameter("gamma_p", [128, ND], F32, isOutput=False)
    beta_p = nc.declare_dram_parameter("beta_p", [128, ND], F32, isOutput=False)
    out = nc.declare_dram_parameter("out", [D, SH], F32, isOutput=True)

    qt8_r = qt8.rearrange("(n p) s -> p n s", p=128)
    qt32_r = qt32.rearrange("(n p) s -> p n s", p=128)
    kt8_r = kt8.rearrange("(n p) t -> p n t", p=128)
    vt8_r = vt8.rearrange("(n p) t -> p n t", p=128)
    wq_r = wq8.rearrange("(n p) c -> p n c", p=128)
    wk_r = wk8.rearrange("(n p) c -> p n c", p=128)
    wv_r = wv8.rearrange("(n p) c -> p n c", p=128)
    wo_r = wo8.rearrange("(n p) c -> p n c", p=128)
    out_r = out.rearrange("(n p) s -> p n s", p=128)

    with tile.TileContext(nc) as tc, ExitStack() as ctx:
        consts = ctx.enter_context(tc.tile_pool(name="consts", bufs=1))
        wpool = ctx.enter_context(tc.tile_pool(name="wpool", bufs=1))
        bigp = ctx.enter_context(tc.tile_pool(name="bigp", bufs=1))
        streams = ctx.enter_context(tc.tile_pool(name="streams", bufs=2))
        work = ctx.enter_context(tc.tile_pool(name="work", bufs=2))
        psum = ctx.enter_context(
            tc.tile_pool(name="psum", bufs=2, space=bass.MemorySpace.PSUM))
        dram = ctx.enter_context(
            tc.tile_pool(name="dram", bufs=1, space="DRAM"))

        # ---- constants ----
        bq_sb = consts.tile([128, ND], F32)
        bk_sb = consts.tile([128, ND], F32)
        gamma_sb = consts.tile([128, ND], F32)
        beta_sb = consts.tile([128, ND], F32)
        bvrow = consts.tile([1, HV], F32)
        bv_bc = consts.tile([128, HV], F32)
        ones8 = consts.tile([128, 2, 64], F8E4)
        stats_p = consts.tile([128, 4 * ND], F32)  # per-sc partials
        stats = consts.tile([128, 2 * ND], F32)
        sqscr = consts.tile([128, 512], F32)
        nc.sync.dma_start(bq_sb[:], bq_p[:])
        nc.sync.dma_start(bk_sb[:], bk_p[:])
        nc.sync.dma_start(gamma_sb[:], gamma_p[:])
        nc.sync.dma_start(beta_sb[:], beta_p[:])
        nc.sync.dma_start(bvrow[:], bv_r[:])
        nc.gpsimd.partition_broadcast(bv_bc[:], bvrow[0:1, :], channels=128)
        nc.vector.memset(ones8[:], 1.0)

        # ---- weights + resident inputs ----
        wk_sb = wpool.tile([128, ND, HV], F8E4, tag="wk")
        wq_sb = wpool.tile([128, ND, HV], F8E4, tag="wq")
        wv_sb = wpool.tile([128, ND, HV], F8E4, tag="wv")
        wo_sb = wpool.tile([128, ND, D], F8E4, tag="wo")
        qt8_sb = wpool.tile([128, ND, SH], F8E4, tag="qt8")
        nc.sync.dma_start(wk_sb[:], wk_r[:])
        nc.sync.dma_start(wq_sb[:], wq_r[:])
        nc.sync.dma_start(qt8_sb[:], qt8_r[:])
        nc.sync.dma_start(wv_sb[:], wv_r[:])
        nc.sync.dma_start(wo_sb[:], wo_r[:])

        # ---- persistent tiles ----
        qT = bigp.tile([128, NPAIR, SH], BF16, tag="qT")
        kT = bigp.tile([128, NPAIR, T], BF16, tag="kT")
        va = bigp.tile([128, TCK, HV], F8E4, tag="va")
        cc = bigp.tile([128, NPAIR, SH], F8E4, tag="cc")
        ot = bigp.tile([128, ND, SH], F32, tag="ot")
        P_all = bigp.tile([128, TCK, SH], F8E4, tag="P")

        # Projection units; each fills one scores-ring psum tile + evacuates.
        def kproj_unit(j, th):
            ps = psum.tile([128, 1024], F32, tag="sc", bufs=3)
            for w in range(2):
                ks = streams.tile([128, ND, 512], F8E4, tag="ks")
                nc.sync.dma_start(ks[:], kt8_r[:, :, ds(th * 1024 + w * 512, 512)])
                for h in range(2):
                    for u in range(4):
                        nc.tensor.matmul(
                            ps[ds(h * 64, 64), ds(w * 512, 512)],
                            wk_sb[:, ds(2 * u, 2), ds(j * 128 + h * 64, 64)],
                            ks[:, ds(2 * u, 2), :],
                            start=(u == 0), stop=(u == 3), perf_mode=PM.DoubleRow)
            nc.vector.tensor_scalar(
                out=kT[:, j, ds(th * 1024, 1024)], in0=ps[:],
                scalar1=INV_W, scalar2=bk_sb[:, ts(j, 1)],
                op0=ALU.mult, op1=ALU.add)

        def qproj_unit(j):
            ps = psum.tile([128, 1024], F32, tag="sc", bufs=3)
            for h in range(2):
                for sc in range(2):
                    for u in range(4):
                        nc.tensor.matmul(
                            ps[ds(h * 64, 64), ds(sc * 512, 512)],
                            wq_sb[:, ds(2 * u, 2), ds(j * 128 + h * 64, 64)],
                            qt8_sb[:, ds(2 * u, 2), ds(sc * 512, 512)],
                            start=(u == 0), stop=(u == 3), perf_mode=PM.DoubleRow)
            nc.scalar.activation(qT[:, j, :], ps[:], AF.Identity,
                                 bias=bq_sb[:, ts(j, 1)], scale=INV_W)

        def vproj_unit(c):
            vs = streams.tile([128, ND, 128], F8E4, tag="vs")
            nc.sync.dma_start(vs[:], vt8_r[:, :, ds(c * 128, 128)])
            ps = psum.tile([128, 1024], F32, tag="sc", bufs=3)
            for h in range(2):
                for w in range(2):
                    for u in range(4):
                        nc.tensor.matmul(
                            ps[ds(h * 64, 64), ds(w * 512, 512)],
                            vs[:, ds(2 * u, 2), ds(h * 64, 64)],
                            wv_sb[:, ds(2 * u, 2), ds(w * 512, 512)],
                            start=(u == 0), stop=(u == 3), perf_mode=PM.DoubleRow)
            # va = psum + 32*bv  (psum is 32*v since wv is prescaled)
            nc.vector.tensor_tensor(out=va[:, c, :], in0=ps[:], in1=bv_bc[:],
                                    op=ALU.add)

        def oproj_unit(d, sc):
            ps = psum.tile([128, 1024], F32, tag="sc", bufs=3)
            for h in range(2):
                for u in range(4):
                    nc.tensor.matmul(
                        ps[ds(h * 64, 64), 0:512],
                        wo_sb[:, ds(2 * u, 2), ds(d * 128 + h * 64, 64)],
                        cc[:, ds(2 * u, 2), ds(sc * 512, 512)],
                        start=(u == 0), stop=(u == 3), perf_mode=PM.DoubleRow)
            qres = streams.tile([128, 512], F32, tag="qres")
            nc.sync.dma_start(qres[:], qt32_r[:, d, ds(sc * 512, 512)])
            seg = ot[:, d, ds(sc * 512, 512)]
            nc.vector.scalar_tensor_tensor(
                out=seg, in0=ps[:, 0:512], scalar=INV_WSQ, in1=qres[:],
                op0=ALU.mult, op1=ALU.add)
            nc.vector.tensor_reduce(out=stats_p[:, ts(sc * 2 * ND + d, 1)],
                                    in_=seg, axis=mybir.AxisListType.X,
                                    op=ALU.add)
            nc.scalar.activation(sqscr[:], seg, AF.Square,
                                 accum_out=stats_p[:, ts(sc * 2 * ND + ND + d, 1)])

        feeder = []

        def pump(k):
            for _ in range(k):
                if feeder:
                    feeder.pop(0)()

        def attn(j, sc, do_vproj):
            vv = psum.tile([128, 512], F32, tag="vv", bufs=1)
            dd = psum.tile([128, 512], F32, tag="dd", bufs=1)
            for c2 in range(NC2):
                if do_vproj:
                    for c in (2 * c2 + 2, 2 * c2 + 3):
                        if c < TCK:
                            vproj_unit(c)
                else:
                    pump(cfg.pump_per_slot)
                for ci in (2 * c2, 2 * c2 + 1):
                    ps = psum.tile([128, 1024], F32, tag="sc", bufs=3)
                    nc.tensor.matmul(ps[:, 0:512],
                                     kT[0:64, j, ds(ci * 128, 128)],
                                     qT[0:64, j, ds(sc * 512, 512)])
                    nc.tensor.matmul(ps[:, 512:1024],
                                     kT[64:128, j, ds(ci * 128, 128)],
                                     qT[64:128, j, ds(sc * 512, 512)])
                    if cfg.exp_pat[ci % 16] == "D":
                        nc.vector.tensor_scalar(
                            out=P_all[:, ci, :].bitcast(I8), in0=ps[:],
                            scalar1=FE_A, scalar2=FE_B,
                            op0=ALU.mult, op1=ALU.add)
                    else:
                        nc.scalar.activation(P_all[:, ci, :], ps[:], AF.Exp,
                                             scale=cfg.scale)
                st, sp = (c2 == 0), (c2 == NC2 - 1)
                nc.tensor.matmul(vv[0:64, :],
                                 va[:, ds(2 * c2, 2), ds(j * 128, 64)],
                                 P_all[:, ds(2 * c2, 2), 0:512],
                                 start=st, stop=sp, perf_mode=PM.DoubleRow)
                nc.tensor.matmul(vv[64:128, :],
                                 va[:, ds(2 * c2, 2), ds(j * 128 + 64, 64)],
                                 P_all[:, ds(2 * c2, 2), 512:1024],
                                 start=st, stop=sp, perf_mode=PM.DoubleRow)
                nc.tensor.matmul(dd[0:1, :], ones8[:, :, 0:1],
                                 P_all[:, ds(2 * c2, 2), 0:512],
                                 start=st, stop=sp, perf_mode=PM.DoubleRow)
                nc.tensor.matmul(dd[64:65, :], ones8[:, :, 0:1],
                                 P_all[:, ds(2 * c2, 2), 512:1024],
                                 start=st, stop=sp, perf_mode=PM.DoubleRow)
            # normalize: cc = vals * (1/den)   (cc ends up as 32*attn_out)
            vvs = work.tile([128, 512], BF16, tag="vvs")
            nc.scalar.activation(vvs[:], vv[:], AF.Copy)
            stg = work.tile([128, 512], F32, tag="stg", bufs=1)
            rcp = work.tile([1, 1024], F32, tag="rcp", bufs=1)
            bc = work.tile([128, 1024], F32, tag="bc")
            nc.scalar.activation(stg[64:65, 0:512], dd[64:65, :], AF.Copy)
            nc.gpsimd.dma_start(stg[0:1, 0:512], stg[64:65, 0:512])
            nc.vector.reciprocal_approx_fast(out=rcp[0:1, 0:512],
                                             in_=dd[0:1, :])
            nc.vector.reciprocal_approx_fast(out=rcp[0:1, 512:1024],
                                             in_=stg[0:1, 0:512])
            nc.gpsimd.partition_broadcast(bc[:], rcp[0:1, :], channels=128)
            nc.vector.tensor_tensor(out=cc[0:64, j, ds(sc * 512, 512)],
                                    in0=vvs[0:64, :], in1=bc[0:64, 0:512],
                                    op=ALU.mult)
            nc.vector.tensor_tensor(out=cc[64:128, j, ds(sc * 512, 512)],
                                    in0=vvs[64:128, :],
                                    in1=bc[64:128, 512:1024], op=ALU.mult)

        # ---- emission ----
        kproj_unit(0, 0)
        kproj_unit(0, 1)
        qproj_unit(0)
        vproj_unit(0)
        vproj_unit(1)
        for sc in range(2):
            for j in range(NPAIR):
                if sc == 0 and j + 1 < NPAIR:
                    feeder.append(lambda j_=j + 1: qproj_unit(j_))
                    feeder.append(lambda j_=j + 1: kproj_unit(j_, 0))
                    feeder.append(lambda j_=j + 1: kproj_unit(j_, 1))
                if sc == 1 and j < 4:
                    feeder.append(lambda d_=2 * j: oproj_unit(d_, 0))
                    feeder.append(lambda d_=2 * j + 1: oproj_unit(d_, 0))
                attn(j, sc, do_vproj=(sc == 0 and j == 0))
                if sc == 0:
                    pump(len(feeder))  # catch up before next pair needs kT/qT
        pump(len(feeder))
        for d in range(ND):
            oproj_unit(d, 1)

        # combine per-sc stat partials
        nc.vector.tensor_tensor(out=stats[:], in0=stats_p[:, 0:2 * ND],
                                in1=stats_p[:, ds(2 * ND, 2 * ND)], op=ALU.add)

        # ---- BN stats allreduce ----
        st_in = dram.tile([128, 2 * ND], F32)
        st_out = dram.tile([128, 2 * ND], F32)
        nc.sync.dma_start(st_in[:], stats[:])
        if cfg.use_collective:
            nc.gpsimd.collective_compute(
                "AllReduce", ALU.add,
                replica_groups=[list(range(cfg.n_cores))],
                ins=[st_in.opt()], outs=[st_out.opt()])
        else:
            nc.sync.dma_start(st_out[:], st_in[:])
        gstats = consts.tile([128, 2 * ND], F32)
        nc.sync.dma_start(gstats[:], st_out[:])

        # ---- BN scale/shift ----
        inv_n = 1.0 / float(cfg.n_total)
        mean = consts.tile([128, ND], F32)
        ex2 = consts.tile([128, ND], F32)
        var = consts.tile([128, ND], F32)
        std = consts.tile([128, ND], F32)
        rstd = consts.tile([128, ND], F32)
        scale_t = consts.tile([128, ND], F32)
        shift_t = consts.tile([128, ND], F32)
        nc.vector.tensor_scalar(out=mean[:], in0=gstats[:, 0:ND],
                                scalar1=inv_n, scalar2=None, op0=ALU.mult)
        nc.vector.tensor_scalar(out=ex2[:], in0=gstats[:, ds(ND, ND)],
                                scalar1=inv_n, scalar2=None, op0=ALU.mult)
        nc.vector.tensor_tensor(out=var[:], in0=mean[:], in1=mean[:], op=ALU.mult)
        nc.vector.tensor_tensor(out=var[:], in0=ex2[:], in1=var[:], op=ALU.subtract)
        nc.vector.tensor_scalar(out=var[:], in0=var[:], scalar1=cfg.eps,
                                scalar2=None, op0=ALU.add)
        nc.scalar.activation(std[:], var[:], AF.Sqrt)
        nc.vector.reciprocal(rstd[:], std[:])
        nc.vector.tensor_tensor(out=scale_t[:], in0=rstd[:], in1=gamma_sb[:],
                                op=ALU.mult)
        nc.vector.tensor_tensor(out=shift_t[:], in0=mean[:], in1=scale_t[:],
                                op=ALU.mult)
        nc.vector.tensor_tensor(out=shift_t[:], in0=beta_sb[:], in1=shift_t[:],
                                op=ALU.subtract)

        # ---- BN apply + output (split across engines) ----
        for d in range(ND):
            fin = work.tile([128, 1024], F32, tag="bc")
            eng = (nc.vector, nc.gpsimd, nc.vector)[d % 3]
            eng.tensor_scalar(out=fin[:], in0=ot[:, d, :],
                              scalar1=scale_t[:, ts(d, 1)],
                              scalar2=shift_t[:, ts(d, 1)],
                              op0=ALU.mult, op1=ALU.add)
            nc.sync.dma_start(out_r[:, d, :], fin[:])

    nc.compile()
    return nc


def prep_core_inputs(cfg, Q, K, V, Wq, bq, Wk, bk, Wv, bv, Wo, bo, gamma, beta,
                     b, half, shared):
    """Build the in_map for core (b, half). Inputs are numpy fp32."""
    D, H, SH = cfg.D, cfg.H, cfg.SH
    ND = cfg.ND
    s0 = half * SH
    key = ("kv", b)
    if key not in shared:
        kt = np.ascontiguousarray(K[b].T)
        vt = np.ascontiguousarray(V[b].T)
        shared[key] = (kt.astype(F8), vt.astype(F8))
    kt8, vt8 = shared[key]
    qt = np.ascontiguousarray(Q[b, s0:s0 + SH, :].T)      # [D, SH]
    return {
        "qt8": qt.astype(F8),
        "qt32": qt + np.asarray(bo, np.float32)[:, None],
        "kt8": kt8, "vt8": vt8,
        "wq8": shared["wq8"], "wk8": shared["wk8"], "wv8": shared["wv8"],
        "wo8": shared["wo8"],
        "bq_p": shared["bq_p"], "bk_p": shared["bk_p"],
        "bv_r": shared["bv_r"],
        "gamma_p": shared["gamma_p"], "beta_p": shared["beta_p"],
    }


_PROGRAM_CACHE = {}


def _get_program(cfg):
    key = (cfg.D, cfg.H, cfg.SH, cfg.T, cfg.n_cores, cfg.exp_pat, cfg.pump_per_slot)
    if key not in _PROGRAM_CACHE:
        _PROGRAM_CACHE[key] = build_program(cfg)
    return _PROGRAM_CACHE[key]


def run(inputs, trace=False, trace_kwargs=None):
    """Run the SPMD kernel; returns (output [B,D,S] fp32, BassKernelResults)."""
    cfg = Cfg()
    args = [np.asarray(inputs[k], np.float32) for k in
            ("Q", "K", "V", "Wq", "bq", "Wk", "bk", "Wv", "bv", "Wo", "bo",
             "gamma", "beta")]
    Q, K, V, Wq, bq, Wk, bk, Wv, bv, Wo, bo, gamma, beta = args
    D, H, ND, ws = cfg.D, cfg.H, cfg.ND, cfg.wscale
    pack = lambda v: np.ascontiguousarray(
        np.asarray(v, np.float32).reshape(ND, 128).T)
    shared = {
        "wq8": (Wq.transpose(1, 0, 2).reshape(D, H * 64) * ws).astype(F8),
        "wk8": (Wk.transpose(1, 0, 2).reshape(D, H * 64) * ws).astype(F8),
        "wv8": (Wv.transpose(1, 0, 2).reshape(D, H * 64) * ws).astype(F8),
        "wo8": (np.asarray(Wo, np.float32) * ws).astype(F8),
        "bq_p": pack(bq), "bk_p": pack(bk),
        "bv_r": (np.asarray(bv, np.float32).reshape(1, H * 64) * ws).copy(),
        "gamma_p": pack(gamma), "beta_p": pack(beta),
    }
    in_maps = [prep_core_inputs(cfg, *args, i // 2, i % 2, shared)
               for i in range(cfg.n_cores)]
    nc = _get_program(cfg)
    res = run_bass_kernel_spmd(nc, in_maps, list(range(cfg.n_cores)),
                               trace=trace, trace_kwargs=trace_kwargs or {})
    B = inputs["Q"].shape[0]
    S = inputs["Q"].shape[1]
    outp = np.empty((B, cfg.D, S), np.float32)
    for i in range(cfg.n_cores):
        b, half = i // 2, i % 2
        outp[b, :, half * cfg.SH:(half + 1) * cfg.SH] = res.results[i]["out"]
    return outp, res


def kernel(**inputs) -> np.ndarray:
    out, _ = run(inputs, trace=False)
    return out


# revision 11
# speedup vs baseline: 1.5015x; 1.0184x over previous
# Trainium2 Bass SPMD kernel for nn_MultiHeadAttn_16492674416882.
#
# kernel(**inputs) takes the FULL fp32 inputs and returns the FULL
# (B, D, S) output, running a fused per-core program on 8 NeuronCores.
#
# Sharding: core i handles batch b=i//2 and query-half h=i%2 (1024 of the
# 2048 positions). K/V projections for a batch are computed by both cores of
# the pair (cheap duplication), which removes every large collective; the
# only cross-core communication is an 8KB AllReduce of BatchNorm statistics.
#
# v2: all projections and the attention*V matmuls run as fp8e4 (E4M3)
# DoubleRow matmuls (2 contraction rows per partition, 2x PE throughput).
# Weights are pre-scaled by 32 on the host so their ~0.02-sigma values sit in
# e4m3's normal range; V output is kept scaled by 32 (va = 32*(v+bv)) so the
# attention output (sigma ~0.01) stays normal in fp8 as well. The score
# matmuls stay bf16 (they are output-rate-bound; fp8 wouldn't help), with
# softmax exp running on the Activation engine (true exp, fp8 out) for most
# tiles and on the DVE for a configurable fraction via the Schraudolph
# bit-trick (x*8/ln2 + 55.68 rounded to int8 == e4m3 bits of exp(x)), which
# balances the two engines. Softmax denominators come from ones-stationary
# DoubleRow matmuls accumulated alongside the attention values in the same
# PSUM tile. The residual is added exactly in fp32 (qt32 = Q^T + bo), the
# pre-BN output stays SBUF-resident, and BatchNorm statistics are
# all-reduced (8KB) before the final scale/shift.

import math
import os
import sys
from contextlib import ExitStack
from dataclasses import dataclass

import numpy as np
import ml_dtypes

for _p in ("/root/.axon_site/_ro/trn_rl_repo", "/opt/trn_rl_repo"):
    if _p not in sys.path and os.path.isdir(_p):
        sys.path.append(_p)

import concourse.bass as bass
import concourse.tile as tile
from concourse import bacc, mybir
from concourse.bass import ds, ts
from concourse.bass_utils import run_bass_kernel_spmd

F32 = mybir.dt.float32
BF16 = mybir.dt.bfloat16
F8E4 = mybir.dt.float8e4
I8 = mybir.dt.int8
AF = mybir.ActivationFunctionType
ALU = mybir.AluOpType
PM = mybir.MatmulPerfMode
BF = ml_dtypes.bfloat16
F8 = ml_dtypes.float8_e4m3


@dataclass
class Cfg:
    D: int = 1024          # model dim (== H*64)
    H: int = 16            # heads
    SH: int = 1024         # queries per core
    T: int = 2048          # kv length
    n_cores: int = 8
    n_total: int = 8192    # BN normalization count (B*S)
    use_collective: bool = True
    eps: float = 1e-5
    scale: float = 1.0 / 32.0    # sqrt(1/1024), exactly 1/32
    wscale: float = 32.0         # fp8 weight prescale
    exp_dve_mod: int = 4         # ci % mod == mod-1 -> exp on DVE bit-trick

    exp_pat: str = "ADADADADADADADAA"  # exp engine per ci%16: A=Act, D=DVE
    pump_per_slot: int = 1       # feeder thunks emitted per c2 slot

    @property
    def ND(self): return self.D // 128
    @property
    def NPAIR(self): return self.H // 2
    @property
    def TCK(self): return self.T // 128     # 128-t chunks
    @property
    def NC2(self): return self.T // 256     # 256-t chunks
    @property
    def HV(self): return self.H * 64


def build_program(cfg: Cfg) -> bass.Bass:
    nc = bacc.Bacc("TRN2", target_bir_lowering=False, debug=False,
                   num_devices=cfg.n_cores)
    D, H, SH, T = cfg.D, cfg.H, cfg.SH, cfg.T
    ND, NPAIR, TCK, NC2 = cfg.ND, cfg.NPAIR, cfg.TCK, cfg.NC2
    HV = cfg.HV
    INV_W = 1.0 / cfg.wscale
    INV_WSQ = 1.0 / (cfg.wscale * cfg.wscale)
    # fast-exp (Schraudolph) constants for e4m3 bits, round-to-nearest on HW
    FE_A = cfg.scale * 8.0 / math.log(2.0)
    FE_B = 56.0 - 0.344

    # ---- I/O ----
    qt8 = nc.declare_dram_parameter("qt8", [D, SH], F8E4, isOutput=False)
    qt32 = nc.declare_dram_parameter("qt32", [D, SH], F32, isOutput=False)
    kt8 = nc.declare_dram_parameter("kt8", [D, T], F8E4, isOutput=False)
    vt8 = nc.declare_dram_parameter("vt8", [D, T], F8E4, isOutput=False)
    wq8 = nc.declare_dram_parameter("wq8", [D, HV], F8E4, isOutput=False)
    wk8 = nc.declare_dram_parameter("wk8", [D, HV], F8E4, isOutput=False)
    wv8 = nc.declare_dram_parameter("wv8", [D, HV], F8E4, isOutput=False)
    wo8 = nc.declare_dram_parameter("wo8", [HV, D], F8E4, isOutput=False)
    bq_p = nc.declare_dram_parameter("bq_p", [128, ND], F32, isOutput=False)
    bk_p = nc.declare_dram_parameter("bk_p", [128, ND], F32, isOutput=False)
    bv_r = nc.declare_dram_parameter("bv_r", [1, HV], F32, isOutput=False)
    gamma_p = nc.declare_dram_parameter("gamma_p", [128, ND], F32, isOutput=False)
    beta_p = nc.declare_dram_parameter("beta_p", [128, ND], F32, isOutput=False)
    out = nc.declare_dram_parameter("out", [D, SH], F32, isOutput=True)

    qt8_r = qt8.rearrange("(n p) s -> p n s", p=128)
    qt32_r = qt32.rearrange("(n p) s -> p n s", p=128)
    kt8_r = kt8.rearrange("(n p) t -> p n t", p=128)
    vt8_r = vt8.rearrange("(n p) t -> p n t", p=128)
    wq_r = wq8.rearrange("(n p) c -> p n c", p=128)
    wk_r = wk8.rearrange("(n p) c -> p n c", p=128)
    wv_r = wv8.rearrange("(n p) c -> p n c", p=128)
    wo_r = wo8.rearrange("(n p) c -> p n c", p=128)
    out_r = out.rearrange("(n p) s -> p n s", p=128)

    with tile.TileContext(nc) as tc, ExitStack() as ctx:
        consts = ctx.enter_context(tc.tile_pool(name="consts", bufs=1))
        wpool = ctx.enter_context(tc.tile_pool(name="wpool", bufs=1))
        bigp = ctx.enter_context(tc.tile_pool(name="bigp", bufs=1))
        streams = ctx.enter_context(tc.tile_pool(name="streams", bufs=2))
        work = ctx.enter_context(tc.tile_pool(name="work", bufs=2))
        psum = ctx.enter_context(
            tc.tile_pool(name="psum", bufs=2, space=bass.MemorySpace.PSUM))
        dram = ctx.enter_context(
            tc.tile_pool(name="dram", bufs=1, space="DRAM"))

        # ---- constants ----
        bq_sb = consts.tile([128, ND], F32)
        bk_sb = consts.tile([128, ND], F32)
        gamma_sb = consts.tile([128, ND], F32)
        beta_sb = consts.tile([128, ND], F32)
        bvrow = consts.tile([1, HV], F32)
        bv_bc = consts.tile([128, HV], F32)
        ones8 = consts.tile([128, 2, 64], F8E4)
        stats_p = consts.tile([128, 4 * ND], F32)  # per-sc partials
        stats = consts.tile([128, 2 * ND], F32)
        sqscr = consts.tile([128, 512], F32)
        nc.sync.dma_start(bq_sb[:], bq_p[:])
        nc.sync.dma_start(bk_sb[:], bk_p[:])
        nc.sync.dma_start(gamma_sb[:], gamma_p[:])
        nc.sync.dma_start(beta_sb[:], beta_p[:])
        nc.sync.dma_start(bvrow[:], bv_r[:])
        nc.gpsimd.partition_broadcast(bv_bc[:], bvrow[0:1, :], channels=128)
        nc.vector.memset(ones8[:], 1.0)

        # ---- weights + resident inputs ----
        wk_sb = wpool.tile([128, ND, HV], F8E4, tag="wk")
        wq_sb = wpool.tile([128, ND, HV], F8E4, tag="wq")
        wv_sb = wpool.tile([128, ND, HV], F8E4, tag="wv")
        wo_sb = wpool.tile([128, ND, D], F8E4, tag="wo")
        qt8_sb = wpool.tile([128, ND, SH], F8E4, tag="qt8")
        nc.sync.dma_start(wk_sb[:], wk_r[:])

        # ---- persistent tiles ----
        qT = bigp.tile([128, NPAIR, SH], BF16, tag="qT")
        kT = bigp.tile([128, NPAIR, T], BF16, tag="kT")
        va = bigp.tile([128, TCK, HV], F8E4, tag="va")
        cc = bigp.tile([128, NPAIR, SH], F8E4, tag="cc")
        ot = bigp.tile([128, ND, SH], F32, tag="ot")
        P_all = bigp.tile([128, TCK, SH], F8E4, tag="P")

        # Projection units; each fills one scores-ring psum tile + evacuates.
        def kproj_unit(j, th):
            ps = psum.tile([128, 1024], F32, tag="sc", bufs=3)
            for w in range(2):
                ks = streams.tile([128, ND, 512], F8E4, tag="ks")
                nc.sync.dma_start(ks[:], kt8_r[:, :, ds(th * 1024 + w * 512, 512)])
                for h in range(2):
                    for u in range(4):
                        nc.tensor.matmul(
                            ps[ds(h * 64, 64), ds(w * 512, 512)],
                            wk_sb[:, ds(2 * u, 2), ds(j * 128 + h * 64, 64)],
                            ks[:, ds(2 * u, 2), :],
                            start=(u == 0), stop=(u == 3), perf_mode=PM.DoubleRow)
            nc.scalar.activation(kT[:, j, ds(th * 1024, 1024)], ps[:],
                                 AF.Identity, bias=bk_sb[:, ts(j, 1)],
                                 scale=INV_W)

        def qproj_unit(j):
            ps = psum.tile([128, 1024], F32, tag="sc", bufs=3)
            for h in range(2):
                for sc in range(2):
                    for u in range(4):
                        nc.tensor.matmul(
                            ps[ds(h * 64, 64), ds(sc * 512, 512)],
                            wq_sb[:, ds(2 * u, 2), ds(j * 128 + h * 64, 64)],
                            qt8_sb[:, ds(2 * u, 2), ds(sc * 512, 512)],
                            start=(u == 0), stop=(u == 3), perf_mode=PM.DoubleRow)
            nc.scalar.activation(qT[:, j, :], ps[:], AF.Identity,
                                 bias=bq_sb[:, ts(j, 1)], scale=INV_W)

        def vproj_unit(c):
            vs = streams.tile([128, ND, 128], F8E4, tag="vs")
            nc.sync.dma_start(vs[:], vt8_r[:, :, ds(c * 128, 128)])
            ps = psum.tile([128, 1024], F32, tag="sc", bufs=3)
            for h in range(2):
                for w in range(2):
                    for u in range(4):
                        nc.tensor.matmul(
                            ps[ds(h * 64, 64), ds(w * 512, 512)],
                            vs[:, ds(2 * u, 2), ds(h * 64, 64)],
                            wv_sb[:, ds(2 * u, 2), ds(w * 512, 512)],
                            start=(u == 0), stop=(u == 3), perf_mode=PM.DoubleRow)
            # va = psum + 32*bv  (psum is 32*v since wv is prescaled)
            nc.vector.tensor_tensor(out=va[:, c, :], in0=ps[:], in1=bv_bc[:],
                                    op=ALU.add)

        def oproj_unit(d, sc):
            ps = psum.tile([128, 1024], F32, tag="sc", bufs=3)
            for h in range(2):
                for u in range(4):
                    nc.tensor.matmul(
                        ps[ds(h * 64, 64), 0:512],
                        wo_sb[:, ds(2 * u, 2), ds(d * 128 + h * 64, 64)],
                        cc[:, ds(2 * u, 2), ds(sc * 512, 512)],
                        start=(u == 0), stop=(u == 3), perf_mode=PM.DoubleRow)
            qres = streams.tile([128, 512], F32, tag="qres")
            nc.sync.dma_start(qres[:], qt32_r[:, d, ds(sc * 512, 512)])
            seg = ot[:, d, ds(sc * 512, 512)]
            nc.vector.scalar_tensor_tensor(
                out=seg, in0=ps[:, 0:512], scalar=INV_WSQ, in1=qres[:],
                op0=ALU.mult, op1=ALU.add)
            nc.vector.tensor_reduce(out=stats_p[:, ts(sc * 2 * ND + d, 1)],
                                    in_=seg, axis=mybir.AxisListType.X,
                                    op=ALU.add)
            nc.scalar.activation(sqscr[:], seg, AF.Square,
                                 accum_out=stats_p[:, ts(sc * 2 * ND + ND + d, 1)])

        feeder = []

        def pump(k):
            for _ in range(k):
                if feeder:
                    feeder.pop(0)()

        def attn(j, sc, do_vproj):
            vv = psum.tile([128, 512], F32, tag="vv", bufs=1)
            dd = psum.tile([128, 512], F32, tag="dd", bufs=1)
            for c2 in range(NC2):
                if do_vproj:
                    for c in (2 * c2 + 2, 2 * c2 + 3):
                        if c < TCK:
                            vproj_unit(c)
                else:
                    pump(cfg.pump_per_slot)
                for ci in (2 * c2, 2 * c2 + 1):
                    ps = psum.tile([128, 1024], F32, tag="sc", bufs=3)
                    nc.tensor.matmul(ps[:, 0:512],
                                     kT[0:64, j, ds(ci * 128, 128)],
                                     qT[0:64, j, ds(sc * 512, 512)])
                    nc.tensor.matmul(ps[:, 512:1024],
                                     kT[64:128, j, ds(ci * 128, 128)],
                                     qT[64:128, j, ds(sc * 512, 512)])
                    if cfg.exp_pat[ci % 16] == "D":
                        nc.vector.tensor_scalar(
                            out=P_all[:, ci, :].bitcast(I8), in0=ps[:],
                            scalar1=FE_A, scalar2=FE_B,
                            op0=ALU.mult, op1=ALU.add)
                    else:
                        nc.scalar.activation(P_all[:, ci, :], ps[:], AF.Exp,
                                             scale=cfg.scale)
                st, sp = (c2 == 0), (c2 == NC2 - 1)
                nc.tensor.matmul(vv[0:64, :],
                                 va[:, ds(2 * c2, 2), ds(j * 128, 64)],
                                 P_all[:, ds(2 * c2, 2), 0:512],
                                 start=st, stop=sp, perf_mode=PM.DoubleRow)
                nc.tensor.matmul(vv[64:128, :],
                                 va[:, ds(2 * c2, 2), ds(j * 128 + 64, 64)],
                                 P_all[:, ds(2 * c2, 2), 512:1024],
                                 start=st, stop=sp, perf_mode=PM.DoubleRow)
                nc.tensor.matmul(dd[0:1, :], ones8[:, :, 0:1],
                                 P_all[:, ds(2 * c2, 2), 0:512],
                                 start=st, stop=sp, perf_mode=PM.DoubleRow)
                nc.tensor.matmul(dd[64:65, :], ones8[:, :, 0:1],
                                 P_all[:, ds(2 * c2, 2), 512:1024],
                                 start=st, stop=sp, perf_mode=PM.DoubleRow)
            # normalize: cc = vals * (1/den)   (cc ends up as 32*attn_out)
            vvs = work.tile([128, 512], BF16, tag="vvs")
            stg = work.tile([128, 512], F32, tag="stg", bufs=1)
            rcp = work.tile([1, 1024], F32, tag="rcp", bufs=1)
            bc = work.tile([128, 1024], F32, tag="bc")
            nc.scalar.activation(stg[64:65, 0:512], dd[64:65, :], AF.Copy)
            nc.gpsimd.dma_start(stg[0:1, 0:512], stg[64:65, 0:512])
            nc.vector.reciprocal_approx_fast(out=rcp[0:1, 0:512],
                                             in_=dd[0:1, :])
            nc.scalar.activation(vvs[:], vv[:], AF.Copy)
            nc.vector.reciprocal_approx_fast(out=rcp[0:1, 512:1024],
                                             in_=stg[0:1, 0:512])
            nc.gpsimd.partition_broadcast(bc[:], rcp[0:1, :], channels=128)
            nc.vector.tensor_tensor(out=cc[0:64, j, ds(sc * 512, 512)],
                                    in0=vvs[0:64, :], in1=bc[0:64, 0:512],
                                    op=ALU.mult)
            nc.vector.tensor_tensor(out=cc[64:128, j, ds(sc * 512, 512)],
                                    in0=vvs[64:128, :],
                                    in1=bc[64:128, 512:1024], op=ALU.mult)

        # ---- emission ----
        kproj_unit(0, 0)
        nc.sync.dma_start(wq_sb[:], wq_r[:])
        nc.sync.dma_start(qt8_sb[:], qt8_r[:])
        kproj_unit(0, 1)
        qproj_unit(0)
        nc.sync.dma_start(wv_sb[:], wv_r[:])
        vproj_unit(0)
        vproj_unit(1)
        for sc in range(2):
            if sc == 1:
                nc.sync.dma_start(wo_sb[:], wo_r[:])
            for j in range(NPAIR):
                if sc == 0 and j + 1 < NPAIR:
                    feeder.append(lambda j_=j + 1: qproj_unit(j_))
                    feeder.append(lambda j_=j + 1: kproj_unit(j_, 0))
                    feeder.append(lambda j_=j + 1: kproj_unit(j_, 1))
                if sc == 1:
                    feeder.append(lambda d_=j: oproj_unit(d_, 0))
                attn(j, sc, do_vproj=(sc == 0 and j == 0))
                if sc == 0:
                    pump(len(feeder))  # catch up before next pair needs kT/qT
        pump(len(feeder))
        for d in range(ND):
            oproj_unit(d, 1)

        # combine per-sc stat partials
        nc.vector.tensor_tensor(out=stats[:], in0=stats_p[:, 0:2 * ND],
                                in1=stats_p[:, ds(2 * ND, 2 * ND)], op=ALU.add)

        # ---- BN stats allreduce ----
        st_in = dram.tile([128, 2 * ND], F32)
        st_out = dram.tile([128, 2 * ND], F32)
        nc.sync.dma_start(st_in[:], stats[:])
        if cfg.use_collective:
            nc.gpsimd.collective_compute(
                "AllReduce", ALU.add,
                replica_groups=[list(range(cfg.n_cores))],
                ins=[st_in.opt()], outs=[st_out.opt()])
        else:
            nc.sync.dma_start(st_out[:], st_in[:])
        gstats = consts.tile([128, 2 * ND], F32)
        nc.sync.dma_start(gstats[:], st_out[:])

        # ---- BN scale/shift ----
        inv_n = 1.0 / float(cfg.n_total)
        mean = consts.tile([128, ND], F32)
        ex2 = consts.tile([128, ND], F32)
        var = consts.tile([128, ND], F32)
        std = consts.tile([128, ND], F32)
        rstd = consts.tile([128, ND], F32)
        scale_t = consts.tile([128, ND], F32)
        shift_t = consts.tile([128, ND], F32)
        nc.vector.tensor_scalar(out=mean[:], in0=gstats[:, 0:ND],
                                scalar1=inv_n, scalar2=None, op0=ALU.mult)
        nc.vector.tensor_scalar(out=ex2[:], in0=gstats[:, ds(ND, ND)],
                                scalar1=inv_n, scalar2=None, op0=ALU.mult)
        nc.vector.tensor_tensor(out=var[:], in0=mean[:], in1=mean[:], op=ALU.mult)
        nc.vector.tensor_tensor(out=var[:], in0=ex2[:], in1=var[:], op=ALU.subtract)
        nc.vector.tensor_scalar(out=var[:], in0=var[:], scalar1=cfg.eps,
                                scalar2=None, op0=ALU.add)
        nc.scalar.activation(std[:], var[:], AF.Sqrt)
        nc.vector.reciprocal(rstd[:], std[:])
        nc.vector.tensor_tensor(out=scale_t[:], in0=rstd[:], in1=gamma_sb[:],
                                op=ALU.mult)
        nc.vector.tensor_tensor(out=shift_t[:], in0=mean[:], in1=scale_t[:],
                                op=ALU.mult)
        nc.vector.tensor_tensor(out=shift_t[:], in0=beta_sb[:], in1=shift_t[:],
                                op=ALU.subtract)

        # ---- BN apply + output (split across engines) ----
        for d in range(ND):
            fin = work.tile([128, 1024], F32, tag="bc")
            eng = (nc.vector, nc.gpsimd, nc.vector)[d % 3]
            eng.tensor_scalar(out=fin[:], in0=ot[:, d, :],
                              scalar1=scale_t[:, ts(d, 1)],
                              scalar2=shift_t[:, ts(d, 1)],
                              op0=ALU.mult, op1=ALU.add)
            nc.sync.dma_start(out_r[:, d, :], fin[:])

    nc.compile()
    return nc


def prep_core_inputs(cfg, Q, K, V, Wq, bq, Wk, bk, Wv, bv, Wo, bo, gamma, beta,
                     b, half, shared):
    """Build the in_map for core (b, half). Inputs are numpy fp32."""
    D, H, SH = cfg.D, cfg.H, cfg.SH
    ND = cfg.ND
    s0 = half * SH
    key = ("kv", b)
    if key not in shared:
        kt = np.ascontiguousarray(K[b].T)
        vt = np.ascontiguousarray(V[b].T)
        shared[key] = (kt.astype(F8), vt.astype(F8))
    kt8, vt8 = shared[key]
    qt = np.ascontiguousarray(Q[b, s0:s0 + SH, :].T)      # [D, SH]
    return {
        "qt8": qt.astype(F8),
        "qt32": qt + np.asarray(bo, np.float32)[:, None],
        "kt8": kt8, "vt8": vt8,
        "wq8": shared["wq8"], "wk8": shared["wk8"], "wv8": shared["wv8"],
        "wo8": shared["wo8"],
        "bq_p": shared["bq_p"], "bk_p": shared["bk_p"],
        "bv_r": shared["bv_r"],
        "gamma_p": shared["gamma_p"], "beta_p": shared["beta_p"],
    }


_PROGRAM_CACHE = {}


def _get_program(cfg):
    key = (cfg.D, cfg.H, cfg.SH, cfg.T, cfg.n_cores, cfg.exp_pat, cfg.pump_per_slot)
    if key not in _PROGRAM_CACHE:
        _PROGRAM_CACHE[key] = build_program(cfg)
    return _PROGRAM_CACHE[key]


def run(inputs, trace=False, trace_kwargs=None):
    """Run the SPMD kernel; returns (output [B,D,S] fp32, BassKernelResults)."""
    cfg = Cfg()
    args = [np.asarray(inputs[k], np.float32) for k in
            ("Q", "K", "V", "Wq", "bq", "Wk", "bk", "Wv", "bv", "Wo", "bo",
             "gamma", "beta")]
    Q, K, V, Wq, bq, Wk, bk, Wv, bv, Wo, bo, gamma, beta = args
    D, H, ND, ws = cfg.D, cfg.H, cfg.ND, cfg.wscale
    pack = lambda v: np.ascontiguousarray(
        np.asarray(v, np.float32).reshape(ND, 128).T)
    shared = {
        "wq8": (Wq.transpose(1, 0, 2).reshape(D, H * 64) * ws).astype(F8),
        "wk8": (Wk.transpose(1, 0, 2).reshape(D, H * 64) * ws).astype(F8),
        "wv8": (Wv.transpose(1, 0, 2).reshape(D, H * 64) * ws).astype(F8),
        "wo8": (np.asarray(Wo, np.float32) * ws).astype(F8),
        "bq_p": pack(bq), "bk_p": pack(bk),
        "bv_r": (np.asarray(bv, np.float32).reshape(1, H * 64) * ws).copy(),
        "gamma_p": pack(gamma), "beta_p": pack(beta),
    }
    in_maps = [prep_core_inputs(cfg, *args, i // 2, i % 2, shared)
               for i in range(cfg.n_cores)]
    nc = _get_program(cfg)
    res = run_bass_kernel_spmd(nc, in_maps, list(range(cfg.n_cores)),
                               trace=trace, trace_kwargs=trace_kwargs or {})
    B = inputs["Q"].shape[0]
    S = inputs["Q"].shape[1]
    outp = np.empty((B, cfg.D, S), np.float32)
    for i in range(cfg.n_cores):
        b, half = i // 2, i % 2
        outp[b, :, half * cfg.SH:(half + 1) * cfg.SH] = res.results[i]["out"]
    return outp, res


def kernel(**inputs) -> np.ndarray:
    out, _ = run(inputs, trace=False)
    return out


# revision 12
# speedup vs baseline: 1.5275x; 1.0173x over previous
# Trainium2 Bass SPMD kernel for nn_MultiHeadAttn_16492674416882.
#
# kernel(**inputs) takes the FULL fp32 inputs and returns the FULL
# (B, D, S) output, running a fused per-core program on 8 NeuronCores.
#
# Sharding: core i handles batch b=i//2 and query-half h=i%2 (1024 of the
# 2048 positions). K/V projections for a batch are computed by both cores of
# the pair (cheap duplication), which removes every large collective; the
# only cross-core communication is an 8KB AllReduce of BatchNorm statistics.
#
# v2: all projections and the attention*V matmuls run as fp8e4 (E4M3)
# DoubleRow matmuls (2 contraction rows per partition, 2x PE throughput).
# Weights are pre-scaled by 32 on the host so their ~0.02-sigma values sit in
# e4m3's normal range; V output is kept scaled by 32 (va = 32*(v+bv)) so the
# attention output (sigma ~0.01) stays normal in fp8 as well. The score
# matmuls stay bf16 (they are output-rate-bound; fp8 wouldn't help), with
# softmax exp running on the Activation engine (true exp, fp8 out) for most
# tiles and on the DVE for a configurable fraction via the Schraudolph
# bit-trick (x*8/ln2 + 55.68 rounded to int8 == e4m3 bits of exp(x)), which
# balances the two engines. Softmax denominators come from ones-stationary
# DoubleRow matmuls accumulated alongside the attention values in the same
# PSUM tile. The residual is added exactly in fp32 (qt32 = Q^T + bo), the
# pre-BN output stays SBUF-resident, and BatchNorm statistics are
# all-reduced (8KB) before the final scale/shift.

import math
import os
import sys
from contextlib import ExitStack
from dataclasses import dataclass

import numpy as np
import ml_dtypes

for _p in ("/root/.axon_site/_ro/trn_rl_repo", "/opt/trn_rl_repo"):
    if _p not in sys.path and os.path.isdir(_p):
        sys.path.append(_p)

import concourse.bass as bass
import concourse.tile as tile
from concourse import bacc, mybir
from concourse.bass import ds, ts
from concourse.bass_utils import run_bass_kernel_spmd

F32 = mybir.dt.float32
BF16 = mybir.dt.bfloat16
F8E4 = mybir.dt.float8e4
I8 = mybir.dt.int8
AF = mybir.ActivationFunctionType
ALU = mybir.AluOpType
PM = mybir.MatmulPerfMode
BF = ml_dtypes.bfloat16
F8 = ml_dtypes.float8_e4m3


@dataclass
class Cfg:
    D: int = 1024          # model dim (== H*64)
    H: int = 16            # heads
    SH: int = 1024         # queries per core
    T: int = 2048          # kv length
    n_cores: int = 8
    n_total: int = 8192    # BN normalization count (B*S)
    use_collective: bool = True
    eps: float = 1e-5
    scale: float = 1.0 / 32.0    # sqrt(1/1024), exactly 1/32
    wscale: float = 32.0         # fp8 weight prescale
    exp_dve_mod: int = 4         # ci % mod == mod-1 -> exp on DVE bit-trick

    exp_pat: str = "ADADADADADADADAA"  # exp engine per ci%16: A=Act, D=DVE
    pump_per_slot: int = 1       # feeder thunks emitted per c2 slot

    @property
    def ND(self): return self.D // 128
    @property
    def NPAIR(self): return self.H // 2
    @property
    def TCK(self): return self.T // 128     # 128-t chunks
    @property
    def NC2(self): return self.T // 256     # 256-t chunks
    @property
    def HV(self): return self.H * 64


def build_program(cfg: Cfg) -> bass.Bass:
    nc = bacc.Bacc("TRN2", target_bir_lowering=False, debug=False,
                   num_devices=cfg.n_cores)
    D, H, SH, T = cfg.D, cfg.H, cfg.SH, cfg.T
    ND, NPAIR, TCK, NC2 = cfg.ND, cfg.NPAIR, cfg.TCK, cfg.NC2
    HV = cfg.HV
    INV_W = 1.0 / cfg.wscale
    INV_WSQ = 1.0 / (cfg.wscale * cfg.wscale)
    # fast-exp (Schraudolph) constants for e4m3 bits, round-to-nearest on HW
    FE_A = cfg.scale * 8.0 / math.log(2.0)
    FE_B = 56.0 - 0.344

    # ---- I/O ----
    qt8 = nc.declare_dram_parameter("qt8", [D, SH], F8E4, isOutput=False)
    qt32 = nc.declare_dram_parameter("qt32", [D, SH], F32, isOutput=False)
    kt8 = nc.declare_dram_parameter("kt8", [D, T], F8E4, isOutput=False)
    vt8 = nc.declare_dram_parameter("vt8", [D, T], F8E4, isOutput=False)
    wq8 = nc.declare_dram_parameter("wq8", [D, HV], F8E4, isOutput=False)
    wk8 = nc.declare_dram_parameter("wk8", [D, HV], F8E4, isOutput=False)
    wv8 = nc.declare_dram_parameter("wv8", [D, HV], F8E4, isOutput=False)
    wo8 = nc.declare_dram_parameter("wo8", [HV, D], F8E4, isOutput=False)
    bq_p = nc.declare_dram_parameter("bq_p", [128, ND], F32, isOutput=False)
    bk_p = nc.declare_dram_parameter("bk_p", [128, ND], F32, isOutput=False)
    bv_r = nc.declare_dram_parameter("bv_r", [1, HV], F32, isOutput=False)
    gamma_p = nc.declare_dram_parameter("gamma_p", [128, ND], F32, isOutput=False)
    beta_p = nc.declare_dram_parameter("beta_p", [128, ND], F32, isOutput=False)
    out = nc.declare_dram_parameter("out", [D, SH], F32, isOutput=True)

    qt8_r = qt8.rearrange("(n p) s -> p n s", p=128)
    qt32_r = qt32.rearrange("(n p) s -> p n s", p=128)
    kt8_r = kt8.rearrange("(n p) t -> p n t", p=128)
    vt8_r = vt8.rearrange("(n p) t -> p n t", p=128)
    wq_r = wq8.rearrange("(n p) c -> p n c", p=128)
    wk_r = wk8.rearrange("(n p) c -> p n c", p=128)
    wv_r = wv8.rearrange("(n p) c -> p n c", p=128)
    wo_r = wo8.rearrange("(n p) c -> p n c", p=128)
    out_r = out.rearrange("(n p) s -> p n s", p=128)

    with tile.TileContext(nc) as tc, ExitStack() as ctx:
        consts = ctx.enter_context(tc.tile_pool(name="consts", bufs=1))
        wpool = ctx.enter_context(tc.tile_pool(name="wpool", bufs=1))
        bigp = ctx.enter_context(tc.tile_pool(name="bigp", bufs=1))
        streams = ctx.enter_context(tc.tile_pool(name="streams", bufs=2))
        work = ctx.enter_context(tc.tile_pool(name="work", bufs=2))
        psum = ctx.enter_context(
            tc.tile_pool(name="psum", bufs=2, space=bass.MemorySpace.PSUM))
        dram = ctx.enter_context(
            tc.tile_pool(name="dram", bufs=1, space="DRAM"))

        # ---- constants ----
        bq_sb = consts.tile([128, ND], F32)
        bk_sb = consts.tile([128, ND], F32)
        gamma_sb = consts.tile([128, ND], F32)
        beta_sb = consts.tile([128, ND], F32)
        bvrow = consts.tile([1, HV], F32)
        bv_bc = consts.tile([128, HV], F32)
        ones8 = consts.tile([128, 2, 64], F8E4)
        stats_p = consts.tile([128, 4 * ND], F32)  # per-sc partials
        stats = consts.tile([128, 2 * ND], F32)
        sqscr = consts.tile([128, 512], F32)
        nc.sync.dma_start(bq_sb[:], bq_p[:])
        nc.sync.dma_start(bk_sb[:], bk_p[:])
        nc.sync.dma_start(gamma_sb[:], gamma_p[:])
        nc.sync.dma_start(beta_sb[:], beta_p[:])
        nc.sync.dma_start(bvrow[:], bv_r[:])
        nc.gpsimd.partition_broadcast(bv_bc[:], bvrow[0:1, :], channels=128)
        nc.vector.memset(ones8[:], 1.0)

        # ---- weights + resident inputs ----
        wk_sb = wpool.tile([128, ND, HV], F8E4, tag="wk")
        wq_sb = wpool.tile([128, ND, HV], F8E4, tag="wq")
        wv_sb = wpool.tile([128, ND, HV], F8E4, tag="wv")
        wo_sb = wpool.tile([128, ND, D], F8E4, tag="wo")
        qt8_sb = wpool.tile([128, ND, SH], F8E4, tag="qt8")
        nc.sync.dma_start(wk_sb[:], wk_r[:])

        # ---- persistent tiles ----
        qT = bigp.tile([128, NPAIR, SH], BF16, tag="qT")
        kT = bigp.tile([128, NPAIR, T], BF16, tag="kT")
        va = bigp.tile([128, TCK, HV], F8E4, tag="va")
        cc = bigp.tile([128, NPAIR, SH], F8E4, tag="cc")
        ot = bigp.tile([128, ND, SH], F32, tag="ot")
        P_all = bigp.tile([128, TCK, SH], F8E4, tag="P")

        # Projection units; each fills one scores-ring psum tile + evacuates.
        def kproj_unit(j, th):
            ps = psum.tile([128, 1024], F32, tag="sc", bufs=3)
            for w in range(2):
                ks = streams.tile([128, ND, 512], F8E4, tag="ks")
                nc.sync.dma_start(ks[:], kt8_r[:, :, ds(th * 1024 + w * 512, 512)])
                for h in range(2):
                    for u in range(4):
                        nc.tensor.matmul(
                            ps[ds(h * 64, 64), ds(w * 512, 512)],
                            wk_sb[:, ds(2 * u, 2), ds(j * 128 + h * 64, 64)],
                            ks[:, ds(2 * u, 2), :],
                            start=(u == 0), stop=(u == 3), perf_mode=PM.DoubleRow)
            nc.scalar.activation(kT[:, j, ds(th * 1024, 1024)], ps[:],
                                 AF.Identity, bias=bk_sb[:, ts(j, 1)],
                                 scale=INV_W)

        def qproj_unit(j):
            ps = psum.tile([128, 1024], F32, tag="sc", bufs=3)
            for h in range(2):
                for sc in range(2):
                    for u in range(4):
                        nc.tensor.matmul(
                            ps[ds(h * 64, 64), ds(sc * 512, 512)],
                            wq_sb[:, ds(2 * u, 2), ds(j * 128 + h * 64, 64)],
                            qt8_sb[:, ds(2 * u, 2), ds(sc * 512, 512)],
                            start=(u == 0), stop=(u == 3), perf_mode=PM.DoubleRow)
            nc.scalar.activation(qT[:, j, :], ps[:], AF.Identity,
                                 bias=bq_sb[:, ts(j, 1)], scale=INV_W)

        def vproj_unit(c):
            vs = streams.tile([128, ND, 128], F8E4, tag="vs")
            nc.sync.dma_start(vs[:], vt8_r[:, :, ds(c * 128, 128)])
            ps = psum.tile([128, 1024], F32, tag="sc", bufs=3)
            for h in range(2):
                for w in range(2):
                    for u in range(4):
                        nc.tensor.matmul(
                            ps[ds(h * 64, 64), ds(w * 512, 512)],
                            vs[:, ds(2 * u, 2), ds(h * 64, 64)],
                            wv_sb[:, ds(2 * u, 2), ds(w * 512, 512)],
                            start=(u == 0), stop=(u == 3), perf_mode=PM.DoubleRow)
            # va = psum + 32*bv  (psum is 32*v since wv is prescaled)
            nc.vector.tensor_tensor(out=va[:, c, :], in0=ps[:], in1=bv_bc[:],
                                    op=ALU.add)

        def oproj_unit(d, sc):
            ps = psum.tile([128, 1024], F32, tag="sc", bufs=3)
            for h in range(2):
                for u in range(4):
                    nc.tensor.matmul(
                        ps[ds(h * 64, 64), 0:512],
                        wo_sb[:, ds(2 * u, 2), ds(d * 128 + h * 64, 64)],
                        cc[:, ds(2 * u, 2), ds(sc * 512, 512)],
                        start=(u == 0), stop=(u == 3), perf_mode=PM.DoubleRow)
            qres = streams.tile([128, 512], F32, tag="qres")
            nc.sync.dma_start(qres[:], qt32_r[:, d, ds(sc * 512, 512)])
            seg = ot[:, d, ds(sc * 512, 512)]
            nc.vector.scalar_tensor_tensor(
                out=seg, in0=ps[:, 0:512], scalar=INV_WSQ, in1=qres[:],
                op0=ALU.mult, op1=ALU.add)
            nc.vector.tensor_reduce(out=stats_p[:, ts(sc * 2 * ND + d, 1)],
                                    in_=seg, axis=mybir.AxisListType.X,
                                    op=ALU.add)
            nc.scalar.activation(sqscr[:], seg, AF.Square,
                                 accum_out=stats_p[:, ts(sc * 2 * ND + ND + d, 1)])

        feeder = []

        def pump(k):
            for _ in range(k):
                if feeder:
                    feeder.pop(0)()

        def attn(j, sc, do_vproj):
            vv = psum.tile([128, 512], F32, tag="vv", bufs=1)
            dd = psum.tile([128, 512], F32, tag="dd", bufs=1)

            def vals(c2):
                st, sp = (c2 == 0), (c2 == NC2 - 1)
                nc.tensor.matmul(vv[0:64, :],
                                 va[:, ds(2 * c2, 2), ds(j * 128, 64)],
                                 P_all[:, ds(2 * c2, 2), 0:512],
                                 start=st, stop=sp, perf_mode=PM.DoubleRow)
                nc.tensor.matmul(vv[64:128, :],
                                 va[:, ds(2 * c2, 2), ds(j * 128 + 64, 64)],
                                 P_all[:, ds(2 * c2, 2), 512:1024],
                                 start=st, stop=sp, perf_mode=PM.DoubleRow)
                nc.tensor.matmul(dd[0:1, :], ones8[:, :, 0:1],
                                 P_all[:, ds(2 * c2, 2), 0:512],
                                 start=st, stop=sp, perf_mode=PM.DoubleRow)
                nc.tensor.matmul(dd[64:65, :], ones8[:, :, 0:1],
                                 P_all[:, ds(2 * c2, 2), 512:1024],
                                 start=st, stop=sp, perf_mode=PM.DoubleRow)

            for c2 in range(NC2):
                if do_vproj:
                    for c in (2 * c2 + 2, 2 * c2 + 3):
                        if c < TCK:
                            vproj_unit(c)
                else:
                    pump(cfg.pump_per_slot)
                for ci in (2 * c2, 2 * c2 + 1):
                    ps = psum.tile([128, 1024], F32, tag="sc", bufs=3)
                    nc.tensor.matmul(ps[:, 0:512],
                                     kT[0:64, j, ds(ci * 128, 128)],
                                     qT[0:64, j, ds(sc * 512, 512)])
                    nc.tensor.matmul(ps[:, 512:1024],
                                     kT[64:128, j, ds(ci * 128, 128)],
                                     qT[64:128, j, ds(sc * 512, 512)])
                    if cfg.exp_pat[ci % 16] == "D":
                        nc.vector.tensor_scalar(
                            out=P_all[:, ci, :].bitcast(I8), in0=ps[:],
                            scalar1=FE_A, scalar2=FE_B,
                            op0=ALU.mult, op1=ALU.add)
                    else:
                        nc.scalar.activation(P_all[:, ci, :], ps[:], AF.Exp,
                                             scale=cfg.scale)
                if c2 > 0:
                    vals(c2 - 1)  # one slot behind: exp(c2-1) already done
            pump(1)
            vals(NC2 - 1)
            # normalize: cc = vals * (1/den)   (cc ends up as 32*attn_out)
            vvs = work.tile([128, 512], BF16, tag="vvs")
            stg = work.tile([128, 512], F32, tag="stg", bufs=1)
            rcp = work.tile([1, 1024], F32, tag="rcp", bufs=1)
            bc = work.tile([128, 1024], F32, tag="bc")
            nc.scalar.activation(stg[64:65, 0:512], dd[64:65, :], AF.Copy)
            nc.gpsimd.dma_start(stg[0:1, 0:512], stg[64:65, 0:512])
            nc.vector.reciprocal_approx_fast(out=rcp[0:1, 0:512],
                                             in_=dd[0:1, :])
            nc.scalar.activation(vvs[:], vv[:], AF.Copy)
            nc.vector.reciprocal_approx_fast(out=rcp[0:1, 512:1024],
                                             in_=stg[0:1, 0:512])
            nc.gpsimd.partition_broadcast(bc[:], rcp[0:1, :], channels=128)
            nc.vector.tensor_tensor(out=cc[0:64, j, ds(sc * 512, 512)],
                                    in0=vvs[0:64, :], in1=bc[0:64, 0:512],
                                    op=ALU.mult)
            nc.vector.tensor_tensor(out=cc[64:128, j, ds(sc * 512, 512)],
                                    in0=vvs[64:128, :],
                                    in1=bc[64:128, 512:1024], op=ALU.mult)

        # ---- emission ----
        kproj_unit(0, 0)
        nc.sync.dma_start(wq_sb[:], wq_r[:])
        nc.sync.dma_start(qt8_sb[:], qt8_r[:])
        kproj_unit(0, 1)
        qproj_unit(0)
        nc.sync.dma_start(wv_sb[:], wv_r[:])
        vproj_unit(0)
        vproj_unit(1)
        for sc in range(2):
            if sc == 1:
                nc.sync.dma_start(wo_sb[:], wo_r[:])
            for j in range(NPAIR):
                if sc == 0 and j + 1 < NPAIR:
                    feeder.append(lambda j_=j + 1: qproj_unit(j_))
                    feeder.append(lambda j_=j + 1: kproj_unit(j_, 0))
                    feeder.append(lambda j_=j + 1: kproj_unit(j_, 1))
                if sc == 1:
                    feeder.append(lambda d_=j: oproj_unit(d_, 0))
                attn(j, sc, do_vproj=(sc == 0 and j == 0))
                if sc == 0:
                    pump(len(feeder))  # catch up before next pair needs kT/qT
        pump(len(feeder))
        for d in range(ND):
            oproj_unit(d, 1)

        # combine per-sc stat partials
        nc.vector.tensor_tensor(out=stats[:], in0=stats_p[:, 0:2 * ND],
                                in1=stats_p[:, ds(2 * ND, 2 * ND)], op=ALU.add)

        # ---- BN stats allreduce ----
        st_in = dram.tile([128, 2 * ND], F32)
        st_out = dram.tile([128, 2 * ND], F32)
        nc.sync.dma_start(st_in[:], stats[:])
        if cfg.use_collective:
            nc.gpsimd.collective_compute(
                "AllReduce", ALU.add,
                replica_groups=[list(range(cfg.n_cores))],
                ins=[st_in.opt()], outs=[st_out.opt()])
        else:
            nc.sync.dma_start(st_out[:], st_in[:])
        gstats = consts.tile([128, 2 * ND], F32)
        nc.sync.dma_start(gstats[:], st_out[:])

        # ---- BN scale/shift ----
        inv_n = 1.0 / float(cfg.n_total)
        mean = consts.tile([128, ND], F32)
        ex2 = consts.tile([128, ND], F32)
        var = consts.tile([128, ND], F32)
        std = consts.tile([128, ND], F32)
        rstd = consts.tile([128, ND], F32)
        scale_t = consts.tile([128, ND], F32)
        shift_t = consts.tile([128, ND], F32)
        nc.vector.tensor_scalar(out=mean[:], in0=gstats[:, 0:ND],
                                scalar1=inv_n, scalar2=None, op0=ALU.mult)
        nc.vector.tensor_scalar(out=ex2[:], in0=gstats[:, ds(ND, ND)],
                                scalar1=inv_n, scalar2=None, op0=ALU.mult)
        nc.vector.tensor_tensor(out=var[:], in0=mean[:], in1=mean[:], op=ALU.mult)
        nc.vector.tensor_tensor(out=var[:], in0=ex2[:], in1=var[:], op=ALU.subtract)
        nc.vector.tensor_scalar(out=var[:], in0=var[:], scalar1=cfg.eps,
                                scalar2=None, op0=ALU.add)
        nc.scalar.activation(std[:], var[:], AF.Sqrt)
        nc.vector.reciprocal(rstd[:], std[:])
        nc.vector.tensor_tensor(out=scale_t[:], in0=rstd[:], in1=gamma_sb[:],
                                op=ALU.mult)
        nc.vector.tensor_tensor(out=shift_t[:], in0=mean[:], in1=scale_t[:],
                                op=ALU.mult)
        nc.vector.tensor_tensor(out=shift_t[:], in0=beta_sb[:], in1=shift_t[:],
                                op=ALU.subtract)

        # ---- BN apply + output (split across engines) ----
        for d in range(ND):
            fin = work.tile([128, 1024], F32, tag="bc")
            eng = (nc.vector, nc.gpsimd, nc.vector)[d % 3]
            eng.tensor_scalar(out=fin[:], in0=ot[:, d, :],
                              scalar1=scale_t[:, ts(d, 1)],
                              scalar2=shift_t[:, ts(d, 1)],
                              op0=ALU.mult, op1=ALU.add)
            nc.sync.dma_start(out_r[:, d, :], fin[:])

    nc.compile()
    return nc


def prep_core_inputs(cfg, Q, K, V, Wq, bq, Wk, bk, Wv, bv, Wo, bo, gamma, beta,
                     b, half, shared):
    """Build the in_map for core (b, half). Inputs are numpy fp32."""
    D, H, SH = cfg.D, cfg.H, cfg.SH
    ND = cfg.ND
    s0 = half * SH
    key = ("kv", b)
    if key not in shared:
        kt = np.ascontiguousarray(K[b].T)
        vt = np.ascontiguousarray(V[b].T)
        shared[key] = (kt.astype(F8), vt.astype(F8))
    kt8, vt8 = shared[key]
    qt = np.ascontiguousarray(Q[b, s0:s0 + SH, :].T)      # [D, SH]
    return {
        "qt8": qt.astype(F8),
        "qt32": qt + np.asarray(bo, np.float32)[:, None],
        "kt8": kt8, "vt8": vt8,
        "wq8": shared["wq8"], "wk8": shared["wk8"], "wv8": shared["wv8"],
        "wo8": shared["wo8"],
        "bq_p": shared["bq_p"], "bk_p": shared["bk_p"],
        "bv_r": shared["bv_r"],
        "gamma_p": shared["gamma_p"], "beta_p": shared["beta_p"],
    }


_PROGRAM_CACHE = {}


def _get_program(cfg):
    key = (cfg.D, cfg.H, cfg.SH, cfg.T, cfg.n_cores, cfg.exp_pat, cfg.pump_per_slot)
    if key not in _PROGRAM_CACHE:
        _PROGRAM_CACHE[key] = build_program(cfg)
    return _PROGRAM_CACHE[key]


def run(inputs, trace=False, trace_kwargs=None):
    """Run the SPMD kernel; returns (output [B,D,S] fp32, BassKernelResults)."""
    cfg = Cfg()
    args = [np.asarray(inputs[k], np.float32) for k in
            ("Q", "K", "V", "Wq", "bq", "Wk", "bk", "Wv", "bv", "Wo", "bo",
             "gamma", "beta")]
    Q, K, V, Wq, bq, Wk, bk, Wv, bv, Wo, bo, gamma, beta = args
    D, H, ND, ws = cfg.D, cfg.H, cfg.ND, cfg.wscale
    pack = lambda v: np.ascontiguousarray(
        np.asarray(v, np.float32).reshape(ND, 128).T)
    shared = {
        "wq8": (Wq.transpose(1, 0, 2).reshape(D, H * 64) * ws).astype(F8),
        "wk8": (Wk.transpose(1, 0, 2).reshape(D, H * 64) * ws).astype(F8),
        "wv8": (Wv.transpose(1, 0, 2).reshape(D, H * 64) * ws).astype(F8),
        "wo8": (np.asarray(Wo, np.float32) * ws).astype(F8),
        "bq_p": pack(bq), "bk_p": pack(bk),
        "bv_r": (np.asarray(bv, np.float32).reshape(1, H * 64) * ws).copy(),
        "gamma_p": pack(gamma), "beta_p": pack(beta),
    }
    in_maps = [prep_core_inputs(cfg, *args, i // 2, i % 2, shared)
               for i in range(cfg.n_cores)]
    nc = _get_program(cfg)
    res = run_bass_kernel_spmd(nc, in_maps, list(range(cfg.n_cores)),
                               trace=trace, trace_kwargs=trace_kwargs or {})
    B = inputs["Q"].shape[0]
    S = inputs["Q"].shape[1]
    outp = np.empty((B, cfg.D, S), np.float32)
    for i in range(cfg.n_cores):
        b, half = i // 2, i % 2
        outp[b, :, half * cfg.SH:(half + 1) * cfg.SH] = res.results[i]["out"]
    return outp, res


def kernel(**inputs) -> np.ndarray:
    out, _ = run(inputs, trace=False)
    return out


# revision 13
# speedup vs baseline: 1.5987x; 1.0465x over previous
# Trainium2 Bass SPMD kernel for nn_MultiHeadAttn_16492674416882.
#
# kernel(**inputs) takes the FULL fp32 inputs and returns the FULL
# (B, D, S) output, running a fused per-core program on 8 NeuronCores.
#
# Sharding: core i handles batch b=i//2 and query-half h=i%2 (1024 of the
# 2048 positions). K/V projections for a batch are computed by both cores of
# the pair (cheap duplication), which removes every large collective; the
# only cross-core communication is an 8KB AllReduce of BatchNorm statistics.
#
# v2: all projections and the attention*V matmuls run as fp8e4 (E4M3)
# DoubleRow matmuls (2 contraction rows per partition, 2x PE throughput).
# Weights are pre-scaled by 32 on the host so their ~0.02-sigma values sit in
# e4m3's normal range; V output is kept scaled by 32 (va = 32*(v+bv)) so the
# attention output (sigma ~0.01) stays normal in fp8 as well. The score
# matmuls stay bf16 (they are output-rate-bound; fp8 wouldn't help), with
# softmax exp running on the Activation engine (true exp, fp8 out) for most
# tiles and on the DVE for a configurable fraction via the Schraudolph
# bit-trick (x*8/ln2 + 55.68 rounded to int8 == e4m3 bits of exp(x)), which
# balances the two engines. Softmax denominators come from ones-stationary
# DoubleRow matmuls accumulated alongside the attention values in the same
# PSUM tile. The residual is added exactly in fp32 (qt32 = Q^T + bo), the
# pre-BN output stays SBUF-resident, and BatchNorm statistics are
# all-reduced (8KB) before the final scale/shift.

import math
import os
import sys
from contextlib import ExitStack
from dataclasses import dataclass

import numpy as np
import ml_dtypes

for _p in ("/root/.axon_site/_ro/trn_rl_repo", "/opt/trn_rl_repo"):
    if _p not in sys.path and os.path.isdir(_p):
        sys.path.append(_p)

import concourse.bass as bass
import concourse.tile as tile
from concourse import bacc, mybir
from concourse.bass import ds, ts
from concourse.bass_utils import run_bass_kernel_spmd

F32 = mybir.dt.float32
BF16 = mybir.dt.bfloat16
F8E4 = mybir.dt.float8e4
I8 = mybir.dt.int8
AF = mybir.ActivationFunctionType
ALU = mybir.AluOpType
PM = mybir.MatmulPerfMode
BF = ml_dtypes.bfloat16
F8 = ml_dtypes.float8_e4m3


@dataclass
class Cfg:
    D: int = 1024          # model dim (== H*64)
    H: int = 16            # heads
    SH: int = 1024         # queries per core
    T: int = 2048          # kv length
    n_cores: int = 8
    n_total: int = 8192    # BN normalization count (B*S)
    use_collective: bool = True
    eps: float = 1e-5
    scale: float = 1.0 / 32.0    # sqrt(1/1024), exactly 1/32
    wscale: float = 32.0         # fp8 weight prescale
    exp_dve_mod: int = 4         # ci % mod == mod-1 -> exp on DVE bit-trick

    exp_pat: str = "ADADADADADADADAA"  # exp engine per ci%16: A=Act, D=DVE
    pump_per_slot: int = 1       # feeder thunks emitted per c2 slot

    @property
    def ND(self): return self.D // 128
    @property
    def NPAIR(self): return self.H // 2
    @property
    def TCK(self): return self.T // 128     # 128-t chunks
    @property
    def NC2(self): return self.T // 256     # 256-t chunks
    @property
    def HV(self): return self.H * 64


def build_program(cfg: Cfg) -> bass.Bass:
    nc = bacc.Bacc("TRN2", target_bir_lowering=False, debug=False,
                   num_devices=cfg.n_cores)
    D, H, SH, T = cfg.D, cfg.H, cfg.SH, cfg.T
    ND, NPAIR, TCK, NC2 = cfg.ND, cfg.NPAIR, cfg.TCK, cfg.NC2
    HV = cfg.HV
    INV_W = 1.0 / cfg.wscale
    INV_WSQ = 1.0 / (cfg.wscale * cfg.wscale)
    # fast-exp (Schraudolph) constants for e4m3 bits, round-to-nearest on HW
    FE_A = cfg.scale * 8.0 / math.log(2.0)
    FE_B = 56.0 - 0.344

    # ---- I/O ----
    qt8 = nc.declare_dram_parameter("qt8", [D, SH], F8E4, isOutput=False)
    qt32 = nc.declare_dram_parameter("qt32", [D, SH], F32, isOutput=False)
    kt8 = nc.declare_dram_parameter("kt8", [D, T], F8E4, isOutput=False)
    vt8 = nc.declare_dram_parameter("vt8", [D, T], F8E4, isOutput=False)
    wq8 = nc.declare_dram_parameter("wq8", [D, HV], F8E4, isOutput=False)
    wk8 = nc.declare_dram_parameter("wk8", [D, HV], F8E4, isOutput=False)
    wv8 = nc.declare_dram_parameter("wv8", [D, HV], F8E4, isOutput=False)
    wo8 = nc.declare_dram_parameter("wo8", [HV, D], F8E4, isOutput=False)
    bq_p = nc.declare_dram_parameter("bq_p", [128, ND], F32, isOutput=False)
    bk_p = nc.declare_dram_parameter("bk_p", [128, ND], F32, isOutput=False)
    bv_r = nc.declare_dram_parameter("bv_r", [1, HV], F32, isOutput=False)
    gamma_p = nc.declare_dram_parameter("gamma_p", [128, ND], F32, isOutput=False)
    beta_p = nc.declare_dram_parameter("beta_p", [128, ND], F32, isOutput=False)
    out = nc.declare_dram_parameter("out", [D, SH], F32, isOutput=True)

    qt8_r = qt8.rearrange("(n p) s -> p n s", p=128)
    qt32_r = qt32.rearrange("(n p) s -> p n s", p=128)
    kt8_r = kt8.rearrange("(n p) t -> p n t", p=128)
    vt8_r = vt8.rearrange("(n p) t -> p n t", p=128)
    wq_r = wq8.rearrange("(n p) c -> p n c", p=128)
    wk_r = wk8.rearrange("(n p) c -> p n c", p=128)
    wv_r = wv8.rearrange("(n p) c -> p n c", p=128)
    wo_r = wo8.rearrange("(n p) c -> p n c", p=128)
    out_r = out.rearrange("(n p) s -> p n s", p=128)

    with tile.TileContext(nc) as tc, ExitStack() as ctx:
        consts = ctx.enter_context(tc.tile_pool(name="consts", bufs=1))
        wpool = ctx.enter_context(tc.tile_pool(name="wpool", bufs=1))
        bigp = ctx.enter_context(tc.tile_pool(name="bigp", bufs=1))
        streams = ctx.enter_context(tc.tile_pool(name="streams", bufs=2))
        work = ctx.enter_context(tc.tile_pool(name="work", bufs=2))
        psum = ctx.enter_context(
            tc.tile_pool(name="psum", bufs=2, space=bass.MemorySpace.PSUM))
        dram = ctx.enter_context(
            tc.tile_pool(name="dram", bufs=1, space="DRAM"))

        # ---- constants ----
        bq_sb = consts.tile([128, ND], F32)
        bk_sb = consts.tile([128, ND], F32)
        gamma_sb = consts.tile([128, ND], F32)
        beta_sb = consts.tile([128, ND], F32)
        bvrow = consts.tile([1, HV], F32)
        bv_bc = consts.tile([128, HV], F32)
        ones8 = consts.tile([128, 2, 64], F8E4)
        stats_p = consts.tile([128, 4 * ND], F32)  # per-sc partials
        stats = consts.tile([128, 2 * ND], F32)
        sqscr = consts.tile([128, 512], F32)
        nc.sync.dma_start(bq_sb[:], bq_p[:])
        nc.sync.dma_start(bk_sb[:], bk_p[:])
        nc.sync.dma_start(gamma_sb[:], gamma_p[:])
        nc.sync.dma_start(beta_sb[:], beta_p[:])
        nc.sync.dma_start(bvrow[:], bv_r[:])
        nc.gpsimd.partition_broadcast(bv_bc[:], bvrow[0:1, :], channels=128)
        nc.vector.memset(ones8[:], 1.0)

        # ---- weights + resident inputs ----
        wk_sb = wpool.tile([128, ND, HV], F8E4, tag="wk")
        wq_sb = wpool.tile([128, ND, HV], F8E4, tag="wq")
        wv_sb = wpool.tile([128, ND, HV], F8E4, tag="wv")
        wo_sb = wpool.tile([128, ND, D], F8E4, tag="wo")
        qt8_sb = wpool.tile([128, ND, SH], F8E4, tag="qt8")

        # ---- persistent tiles ----
        qT = bigp.tile([128, NPAIR, SH], BF16, tag="qT")
        kT = bigp.tile([128, NPAIR, T], BF16, tag="kT")
        va = bigp.tile([128, TCK, HV], F8E4, tag="va")
        cc = bigp.tile([128, NPAIR, SH], F8E4, tag="cc")
        ot = bigp.tile([128, ND, SH], F32, tag="ot")
        P_all = bigp.tile([128, TCK, SH], F8E4, tag="P")

        # Projection units; each fills one scores-ring psum tile + evacuates.
        def kproj_unit(j, th):
            if th == 0:
                nc.sync.dma_start(wk_sb[:, :, ds(j * 128, 128)],
                                  wk_r[:, :, ds(j * 128, 128)])
            ps = psum.tile([128, 1024], F32, tag="sc", bufs=3)
            for w in range(2):
                ks = streams.tile([128, ND, 512], F8E4, tag="ks")
                nc.sync.dma_start(ks[:], kt8_r[:, :, ds(th * 1024 + w * 512, 512)])
                for h in range(2):
                    for u in range(4):
                        nc.tensor.matmul(
                            ps[ds(h * 64, 64), ds(w * 512, 512)],
                            wk_sb[:, ds(2 * u, 2), ds(j * 128 + h * 64, 64)],
                            ks[:, ds(2 * u, 2), :],
                            start=(u == 0), stop=(u == 3), perf_mode=PM.DoubleRow)
            nc.scalar.activation(kT[:, j, ds(th * 1024, 1024)], ps[:],
                                 AF.Identity, bias=bk_sb[:, ts(j, 1)],
                                 scale=INV_W)

        def qproj_unit(j):
            nc.sync.dma_start(wq_sb[:, :, ds(j * 128, 128)],
                              wq_r[:, :, ds(j * 128, 128)])
            ps = psum.tile([128, 1024], F32, tag="sc", bufs=3)
            for h in range(2):
                for sc in range(2):
                    for u in range(4):
                        nc.tensor.matmul(
                            ps[ds(h * 64, 64), ds(sc * 512, 512)],
                            wq_sb[:, ds(2 * u, 2), ds(j * 128 + h * 64, 64)],
                            qt8_sb[:, ds(2 * u, 2), ds(sc * 512, 512)],
                            start=(u == 0), stop=(u == 3), perf_mode=PM.DoubleRow)
            nc.scalar.activation(qT[:, j, :], ps[:], AF.Identity,
                                 bias=bq_sb[:, ts(j, 1)], scale=INV_W)

        def vproj_unit(c):
            vs = streams.tile([128, ND, 128], F8E4, tag="vs")
            nc.sync.dma_start(vs[:], vt8_r[:, :, ds(c * 128, 128)])
            ps = psum.tile([128, 1024], F32, tag="sc", bufs=3)
            for h in range(2):
                for w in range(2):
                    for u in range(4):
                        nc.tensor.matmul(
                            ps[ds(h * 64, 64), ds(w * 512, 512)],
                            vs[:, ds(2 * u, 2), ds(h * 64, 64)],
                            wv_sb[:, ds(2 * u, 2), ds(w * 512, 512)],
                            start=(u == 0), stop=(u == 3), perf_mode=PM.DoubleRow)
            # va = psum + 32*bv  (psum is 32*v since wv is prescaled)
            nc.vector.tensor_tensor(out=va[:, c, :], in0=ps[:], in1=bv_bc[:],
                                    op=ALU.add)

        def oproj_unit(d, sc):
            ps = psum.tile([128, 1024], F32, tag="sc", bufs=3)
            for h in range(2):
                for u in range(4):
                    nc.tensor.matmul(
                        ps[ds(h * 64, 64), 0:512],
                        wo_sb[:, ds(2 * u, 2), ds(d * 128 + h * 64, 64)],
                        cc[:, ds(2 * u, 2), ds(sc * 512, 512)],
                        start=(u == 0), stop=(u == 3), perf_mode=PM.DoubleRow)
            qres = streams.tile([128, 512], F32, tag="qres")
            nc.sync.dma_start(qres[:], qt32_r[:, d, ds(sc * 512, 512)])
            seg = ot[:, d, ds(sc * 512, 512)]
            nc.vector.scalar_tensor_tensor(
                out=seg, in0=ps[:, 0:512], scalar=INV_WSQ, in1=qres[:],
                op0=ALU.mult, op1=ALU.add,
                accum_out=stats_p[:, ts(sc * 2 * ND + d, 1)])
            nc.scalar.activation(sqscr[:], seg, AF.Square,
                                 accum_out=stats_p[:, ts(sc * 2 * ND + ND + d, 1)])

        feeder = []

        def pump(k):
            for _ in range(k):
                if feeder:
                    feeder.pop(0)()

        def attn(j, sc, do_vproj):
            vv = psum.tile([128, 512], F32, tag="vv", bufs=1)
            dd = psum.tile([128, 512], F32, tag="dd", bufs=1)

            def vals(c2):
                st, sp = (c2 == 0), (c2 == NC2 - 1)
                nc.tensor.matmul(vv[0:64, :],
                                 va[:, ds(2 * c2, 2), ds(j * 128, 64)],
                                 P_all[:, ds(2 * c2, 2), 0:512],
                                 start=st, stop=sp, perf_mode=PM.DoubleRow)
                nc.tensor.matmul(vv[64:128, :],
                                 va[:, ds(2 * c2, 2), ds(j * 128 + 64, 64)],
                                 P_all[:, ds(2 * c2, 2), 512:1024],
                                 start=st, stop=sp, perf_mode=PM.DoubleRow)
                nc.tensor.matmul(dd[0:1, :], ones8[:, :, 0:1],
                                 P_all[:, ds(2 * c2, 2), 0:512],
                                 start=st, stop=sp, perf_mode=PM.DoubleRow)
                nc.tensor.matmul(dd[64:65, :], ones8[:, :, 0:1],
                                 P_all[:, ds(2 * c2, 2), 512:1024],
                                 start=st, stop=sp, perf_mode=PM.DoubleRow)

            for c2 in range(NC2):
                if do_vproj:
                    for c in (2 * c2 + 2, 2 * c2 + 3):
                        if c < TCK:
                            vproj_unit(c)
                else:
                    pump(cfg.pump_per_slot)
                for ci in (2 * c2, 2 * c2 + 1):
                    ps = psum.tile([128, 1024], F32, tag="sc", bufs=3)
                    nc.tensor.matmul(ps[:, 0:512],
                                     kT[0:64, j, ds(ci * 128, 128)],
                                     qT[0:64, j, ds(sc * 512, 512)])
                    nc.tensor.matmul(ps[:, 512:1024],
                                     kT[64:128, j, ds(ci * 128, 128)],
                                     qT[64:128, j, ds(sc * 512, 512)])
                    if cfg.exp_pat[ci % 16] == "D":
                        nc.vector.tensor_scalar(
                            out=P_all[:, ci, :].bitcast(I8), in0=ps[:],
                            scalar1=FE_A, scalar2=FE_B,
                            op0=ALU.mult, op1=ALU.add)
                    else:
                        nc.scalar.activation(P_all[:, ci, :], ps[:], AF.Exp,
                                             scale=cfg.scale)
                if c2 > 0:
                    vals(c2 - 1)  # one slot behind: exp(c2-1) already done
            pump(1)
            vals(NC2 - 1)
            # normalize: cc = vals * (1/den)   (cc ends up as 32*attn_out)
            vvs = work.tile([128, 512], BF16, tag="vvs")
            stg = work.tile([128, 512], F32, tag="stg", bufs=1)
            rcp = work.tile([1, 1024], F32, tag="rcp", bufs=1)
            bc = work.tile([128, 1024], F32, tag="bc")
            nc.scalar.activation(stg[64:65, 0:512], dd[64:65, :], AF.Copy)
            nc.gpsimd.dma_start(stg[0:1, 0:512], stg[64:65, 0:512])
            nc.vector.reciprocal_approx_fast(out=rcp[0:1, 0:512],
                                             in_=dd[0:1, :])
            nc.scalar.activation(vvs[:], vv[:], AF.Copy)
            nc.vector.reciprocal_approx_fast(out=rcp[0:1, 512:1024],
                                             in_=stg[0:1, 0:512])
            nc.gpsimd.partition_broadcast(bc[:], rcp[0:1, :], channels=128)
            nc.gpsimd.tensor_tensor(out=cc[0:64, j, ds(sc * 512, 512)],
                                    in0=vvs[0:64, :], in1=bc[0:64, 0:512],
                                    op=ALU.mult)
            nc.gpsimd.tensor_tensor(out=cc[64:128, j, ds(sc * 512, 512)],
                                    in0=vvs[64:128, :],
                                    in1=bc[64:128, 512:1024], op=ALU.mult)

        # ---- emission ----
        kproj_unit(0, 0)
        nc.sync.dma_start(qt8_sb[:], qt8_r[:])
        kproj_unit(0, 1)
        qproj_unit(0)
        nc.sync.dma_start(wv_sb[:], wv_r[:])
        vproj_unit(0)
        vproj_unit(1)
        for sc in range(2):
            if sc == 1:
                nc.sync.dma_start(wo_sb[:], wo_r[:])
            for j in range(NPAIR):
                if sc == 0 and j + 1 < NPAIR:
                    feeder.append(lambda j_=j + 1: qproj_unit(j_))
                    feeder.append(lambda j_=j + 1: kproj_unit(j_, 0))
                    feeder.append(lambda j_=j + 1: kproj_unit(j_, 1))
                if sc == 1:
                    feeder.append(lambda d_=j: oproj_unit(d_, 0))
                attn(j, sc, do_vproj=(sc == 0 and j == 0))
                if sc == 0:
                    pump(len(feeder))  # catch up before next pair needs kT/qT
        pump(len(feeder))
        for d in range(ND):
            oproj_unit(d, 1)

        # combine per-sc stat partials
        nc.vector.tensor_tensor(out=stats[:], in0=stats_p[:, 0:2 * ND],
                                in1=stats_p[:, ds(2 * ND, 2 * ND)], op=ALU.add)

        # ---- BN stats allreduce ----
        st_in = dram.tile([128, 2 * ND], F32)
        st_out = dram.tile([128, 2 * ND], F32)
        nc.sync.dma_start(st_in[:], stats[:])
        if cfg.use_collective:
            nc.gpsimd.collective_compute(
                "AllReduce", ALU.add,
                replica_groups=[list(range(cfg.n_cores))],
                ins=[st_in.opt()], outs=[st_out.opt()])
        else:
            nc.sync.dma_start(st_out[:], st_in[:])
        gstats = consts.tile([128, 2 * ND], F32)
        nc.sync.dma_start(gstats[:], st_out[:])

        # ---- BN scale/shift ----
        inv_n = 1.0 / float(cfg.n_total)
        mean = consts.tile([128, ND], F32)
        ex2 = consts.tile([128, ND], F32)
        var = consts.tile([128, ND], F32)
        std = consts.tile([128, ND], F32)
        rstd = consts.tile([128, ND], F32)
        scale_t = consts.tile([128, ND], F32)
        shift_t = consts.tile([128, ND], F32)
        nc.vector.tensor_scalar(out=mean[:], in0=gstats[:, 0:ND],
                                scalar1=inv_n, scalar2=None, op0=ALU.mult)
        nc.vector.tensor_scalar(out=ex2[:], in0=gstats[:, ds(ND, ND)],
                                scalar1=inv_n, scalar2=None, op0=ALU.mult)
        nc.vector.tensor_tensor(out=var[:], in0=mean[:], in1=mean[:], op=ALU.mult)
        nc.vector.tensor_tensor(out=var[:], in0=ex2[:], in1=var[:], op=ALU.subtract)
        nc.vector.tensor_scalar(out=var[:], in0=var[:], scalar1=cfg.eps,
                                scalar2=None, op0=ALU.add)
        nc.scalar.activation(std[:], var[:], AF.Sqrt)
        nc.vector.reciprocal(rstd[:], std[:])
        nc.vector.tensor_tensor(out=scale_t[:], in0=rstd[:], in1=gamma_sb[:],
                                op=ALU.mult)
        nc.vector.tensor_tensor(out=shift_t[:], in0=mean[:], in1=scale_t[:],
                                op=ALU.mult)
        nc.vector.tensor_tensor(out=shift_t[:], in0=beta_sb[:], in1=shift_t[:],
                                op=ALU.subtract)

        # ---- BN apply + output (split across engines) ----
        for d in range(ND):
            fin = work.tile([128, 1024], F32, tag="bc")
            eng = (nc.vector, nc.gpsimd, nc.vector)[d % 3]
            eng.tensor_scalar(out=fin[:], in0=ot[:, d, :],
                              scalar1=scale_t[:, ts(d, 1)],
                              scalar2=shift_t[:, ts(d, 1)],
                              op0=ALU.mult, op1=ALU.add)
            nc.sync.dma_start(out_r[:, d, :], fin[:])

    nc.compile()
    return nc


def prep_core_inputs(cfg, Q, K, V, Wq, bq, Wk, bk, Wv, bv, Wo, bo, gamma, beta,
                     b, half, shared):
    """Build the in_map for core (b, half). Inputs are numpy fp32."""
    D, H, SH = cfg.D, cfg.H, cfg.SH
    ND = cfg.ND
    s0 = half * SH
    key = ("kv", b)
    if key not in shared:
        kt = np.ascontiguousarray(K[b].T)
        vt = np.ascontiguousarray(V[b].T)
        shared[key] = (kt.astype(F8), vt.astype(F8))
    kt8, vt8 = shared[key]
    qt = np.ascontiguousarray(Q[b, s0:s0 + SH, :].T)      # [D, SH]
    return {
        "qt8": qt.astype(F8),
        "qt32": qt + np.asarray(bo, np.float32)[:, None],
        "kt8": kt8, "vt8": vt8,
        "wq8": shared["wq8"], "wk8": shared["wk8"], "wv8": shared["wv8"],
        "wo8": shared["wo8"],
        "bq_p": shared["bq_p"], "bk_p": shared["bk_p"],
        "bv_r": shared["bv_r"],
        "gamma_p": shared["gamma_p"], "beta_p": shared["beta_p"],
    }


_PROGRAM_CACHE = {}


def _get_program(cfg):
    key = (cfg.D, cfg.H, cfg.SH, cfg.T, cfg.n_cores, cfg.exp_pat, cfg.pump_per_slot)
    if key not in _PROGRAM_CACHE:
        _PROGRAM_CACHE[key] = build_program(cfg)
    return _PROGRAM_CACHE[key]


def run(inputs, trace=False, trace_kwargs=None):
    """Run the SPMD kernel; returns (output [B,D,S] fp32, BassKernelResults)."""
    cfg = Cfg()
    args = [np.asarray(inputs[k], np.float32) for k in
            ("Q", "K", "V", "Wq", "bq", "Wk", "bk", "Wv", "bv", "Wo", "bo",
             "gamma", "beta")]
    Q, K, V, Wq, bq, Wk, bk, Wv, bv, Wo, bo, gamma, beta = args
    D, H, ND, ws = cfg.D, cfg.H, cfg.ND, cfg.wscale
    pack = lambda v: np.ascontiguousarray(
        np.asarray(v, np.float32).reshape(ND, 128).T)
    shared = {
        "wq8": (Wq.transpose(1, 0, 2).reshape(D, H * 64) * ws).astype(F8),
        "wk8": (Wk.transpose(1, 0, 2).reshape(D, H * 64) * ws).astype(F8),
        "wv8": (Wv.transpose(1, 0, 2).reshape(D, H * 64) * ws).astype(F8),
        "wo8": (np.asarray(Wo, np.float32) * ws).astype(F8),
        "bq_p": pack(bq), "bk_p": pack(bk),
        "bv_r": (np.asarray(bv, np.float32).reshape(1, H * 64) * ws).copy(),
        "gamma_p": pack(gamma), "beta_p": pack(beta),
    }
    in_maps = [prep_core_inputs(cfg, *args, i // 2, i % 2, shared)
               for i in range(cfg.n_cores)]
    nc = _get_program(cfg)
    res = run_bass_kernel_spmd(nc, in_maps, list(range(cfg.n_cores)),
                               trace=trace, trace_kwargs=trace_kwargs or {})
    B = inputs["Q"].shape[0]
    S = inputs["Q"].shape[1]
    outp = np.empty((B, cfg.D, S), np.float32)
    for i in range(cfg.n_cores):
        b, half = i // 2, i % 2
        outp[b, :, half * cfg.SH:(half + 1) * cfg.SH] = res.results[i]["out"]
    return outp, res


def kernel(**inputs) -> np.ndarray:
    out, _ = run(inputs, trace=False)
    return out
